# revision 16
# baseline (speedup 1.0000x reference)
"""Expert-parallel MoE routing kernel for Trainium2 (8 NeuronCores).

Problem: top-k(=2) softmax-gated MoE FFN (relu), followed by
log_softmax(sum(moe_out, axis=-1)) over the sequence dim.

Key algebraic observation: the graded output is
    log_softmax_S( sum_d moe_out[t, d] )
and
    sum_d moe_out[t, :] = sum_e combine[t,e] * (relu(x_t @ W1_e + b1_e) @ rowsum(W2_e) + sum(b2_e))
so the second expert matmul collapses to a matvec against rowsum(W2_e).
All of W2 must still be read from HBM (memory-regime roofline unchanged);
its row-sum is computed on-device by the Vector engine while W1 streams
into the Tensor engine.

Sharding (per the expert-parallel hint): core e owns expert e's weights.
The host computes the (tiny) gate/top-k routing to build the dispatch
(it must, to construct the per-core input shards), gathers each expert's
tokens, and the device does the entire FFN including gate-value scaling.
Host then scatter-adds the per-(token,expert) scalars and applies the
final log_softmax on the [B, S] result.

Matmuls run as float32r (fp32 storage, reduced-precision PE mode, 4x the
throughput of strict fp32; measured rel-err ~2e-4 end to end).
"""

import os

import numpy as np

N_CORES = 8
P = 128


def _round_up(v, m):
    return ((v + m - 1) // m) * m


def _chunks(C):
    if C <= 512:
        return [(0, C)]
    if MM_MODE == "bf16":
        # big matmuls amortize per-instruction + weight-load overhead
        out = []
        off = 0
        while off < C:
            ln = min(512, C - off)
            out.append((off, ln))
            off += ln
        return out
    # float32r needs each chunk >=256 to run at its fast rate
    assert C % 256 == 0
    h = C // 2
    return [(0, h), (h, h)]


_BUILD_CACHE = {}

# matmul operand dtype: "bf16" (1 cy/row, in-flight cast on DMA, ~3e-3 rel err)
# or "f32r" (fp32 storage, ~2.8 cy/row, ~1e-4 rel err)
MM_MODE = os.environ.get("MOE_MM_MODE", "bf16")


def _build_program(D, H, C, n_b2):
    """Trace + compile the single-core program (SPMD across 8 cores).

    Per-core inputs:
      xtg [D, C]  f32r  gathered tokens for this expert, transposed
      w1  [D, H]  f32r  expert's first-layer weight (natural = lhsT layout)
      b1t [P, H/P] f32  expert's first-layer bias, column m = b1[m*128:(m+1)*128]
      w2  [H, n_b2] f32 expert's second-layer weight
      b2  [1, n_b2] f32 expert's second-layer bias
      g   [1, C]  f32   gate values per slot (0 for padding slots)
    Output:
      z [1, C] f32 = g * (relu(x @ w1 + b1) @ rowsum(w2) + sum(b2))
    """
    key = (D, H, C, n_b2, MM_MODE)
    if key in _BUILD_CACHE:
        return _BUILD_CACHE[key]

    import concourse.tile as tile
    from concourse import bacc, mybir

    f32 = mybir.dt.float32
    mmdt = mybir.dt.bfloat16 if MM_MODE == "bf16" else mybir.dt.float32r
    KD = D // P  # k-tiles over D
    MH = H // P  # m-tiles over H
    chunks = _chunks(C)

    nc = bacc.Bacc("TRN2", target_bir_lowering=False, debug=False)
    in_dt = f32 if MM_MODE == "bf16" else mybir.dt.float32r
    xtg_d = nc.dram_tensor("xtg", [D, C], in_dt, kind="ExternalInput").ap()
    w1_d = nc.dram_tensor("w1", [D, H], in_dt, kind="ExternalInput").ap()
    b1t_d = nc.dram_tensor("b1t", [P, MH], f32, kind="ExternalInput").ap()
    w2_d = nc.dram_tensor("w2", [H, n_b2], f32, kind="ExternalInput").ap()
    b2_d = nc.dram_tensor("b2", [1, n_b2], f32, kind="ExternalInput").ap()
    g_d = nc.dram_tensor("g", [1, C], f32, kind="ExternalInput").ap()
    z_d = nc.dram_tensor("z", [1, C], f32, kind="ExternalOutput").ap()

    cast_needed = MM_MODE == "bf16"

    with tile.TileContext(nc) as tc:
        with (
            tc.tile_pool(name="persist", bufs=1) as persist,
            tc.tile_pool(name="stage", bufs=4) as stage,
            tc.tile_pool(name="w2p", bufs=3) as w2p,
            tc.tile_pool(name="psum_h", bufs=6, space="PSUM") as psum_h,
            tc.tile_pool(name="psum_z", bufs=2, space="PSUM") as psum_z,
        ):
            def load_cast(dst_ap, src_ap, stage_shape, eng=None):
                # fast HWDGE fp32 load + engine cast copy; plain HWDGE load
                # when no cast is needed
                if cast_needed:
                    st = stage.tile(stage_shape, f32, tag="stage", name="stage")
                    nc.sync.dma_start(out=st[:], in_=src_ap)
                    (eng or nc.vector).tensor_copy(out=dst_ap, in_=st[:])
                else:
                    nc.sync.dma_start(out=dst_ap, in_=src_ap)

            # --- activations, transposed+gathered: 8 tiles [128, C] ---
            # issued first: these + w1 gate the Tensor engine's start
            xtg_tiles = []
            for kd in range(KD):
                t = persist.tile([P, C], mmdt, tag=f"xtg{kd}", name=f"xtg{kd}")
                load_cast(t[:], xtg_d[kd * P : (kd + 1) * P, :], [P, C])
                xtg_tiles.append(t)

            # --- w1: resident k-major tiles, loaded in 512-column pieces
            # (= one m-group's worth) so the Tensor engine starts after the
            # first piece instead of the whole 8 MB; casts run on the
            # otherwise-idle GpSimd engine ---
            GRP = 4
            PIECE = GRP * P
            w1_tiles = []
            for kd in range(KD):
                t = persist.tile([P, H], mmdt, tag=f"w1k{kd}", name=f"w1k{kd}")
                w1_tiles.append(t)
            for p0 in range(0, H, PIECE):
                for kd in range(KD):
                    sl = slice(p0, p0 + PIECE)
                    load_cast(
                        w1_tiles[kd][:, sl],
                        w1_d[kd * P : (kd + 1) * P, sl],
                        [P, PIECE],
                        eng=nc.gpsimd,
                    )

            # --- small persistent loads ---
            g_sb = persist.tile([1, C], f32)
            nc.sync.dma_start(out=g_sb[:], in_=g_d[:])
            b1t_sb = persist.tile([P, MH], f32)
            nc.sync.dma_start(out=b1t_sb[:], in_=b1t_d[:])
            b2_sb = persist.tile([1, n_b2], f32)
            nc.sync.dma_start(out=b2_sb[:], in_=b2_d[:])
            b2sum = persist.tile([1, 1], f32)
            nc.vector.reduce_sum(out=b2sum[:], in_=b2_sb[:], axis=mybir.AxisListType.X)

            # w2 row-sums, column m = rowsum over free dim of w2 m-tile
            w2sum = persist.tile([P, MH], mmdt)
            # hT per chunk: [128, MH * chunk_len], slice m holds h^T m-tile
            ht_tiles = [
                persist.tile([P, MH * ln], mmdt, tag=f"ht{ci}", name=f"ht{ci}")
                for ci, (off, ln) in enumerate(chunks)
            ]

            # --- mm1 + relu: h^T[m-tile] = relu(w1^T x^T + b1) ---
            # m-groups of GRP (aligned to the w1 DMA pieces) with the kd
            # accumulation loop outer-per-group: GRP open PSUM accumulators,
            # so the Tensor engine starts on group 0 right after xtg and the
            # first w1 piece land, and both C-chunks run while the group's
            # weights are resident
            for m0 in range(0, MH, GRP):
                grp = range(m0, min(m0 + GRP, MH))
                for ci, (off, ln) in enumerate(chunks):
                    pss = {}
                    for m in grp:
                        pss[m] = psum_h.tile([P, ln], f32, tag="psh", name="psh")
                    for kd in range(KD):
                        for m in grp:
                            nc.tensor.matmul(
                                pss[m][:],
                                w1_tiles[kd][:, m * P : (m + 1) * P],
                                xtg_tiles[kd][:, off : off + ln],
                                start=(kd == 0),
                                stop=(kd == KD - 1),
                            )
                    for m in grp:
                        nc.scalar.activation(
                            ht_tiles[ci][:, m * ln : (m + 1) * ln],
                            pss[m][:],
                            mybir.ActivationFunctionType.Relu,
                            bias=b1t_sb[:, m : m + 1],
                        )

            # --- w2 stream (DVE row-sums, overlaps the PE loop) ---
            # issued on the sync queue AFTER w1/xtg so it does not steal
            # early HBM bandwidth; w2sum is only needed by the matvec at
            # the very end
            for m in range(MH):
                w2t = w2p.tile([P, n_b2], f32)
                nc.sync.dma_start(out=w2t[:], in_=w2_d[m * P : (m + 1) * P, :])
                with nc.allow_low_precision(
                    reason="w2 row-sum feeds reduced-precision matmul anyway"
                ):
                    nc.vector.reduce_sum(
                        out=w2sum[:, m : m + 1], in_=w2t[:], axis=mybir.AxisListType.X
                    )

            # --- matvec against w2 row-sums + bias + gate scale ---
            z_sb = persist.tile([1, C], f32)
            for ci, (off, ln) in enumerate(chunks):
                pz = psum_z.tile([1, ln], f32)
                for m in range(MH):
                    nc.tensor.matmul(
                        pz[:],
                        w2sum[:, m : m + 1],
                        ht_tiles[ci][:, m * ln : (m + 1) * ln],
                        start=(m == 0),
                        stop=(m == MH - 1),
                    )
                nc.scalar.activation(
                    z_sb[:, off : off + ln],
                    pz[:],
                    mybir.ActivationFunctionType.Identity,
                    bias=b2sum[:],
                )
                nc.vector.tensor_mul(
                    z_sb[:, off : off + ln],
                    z_sb[:, off : off + ln],
                    g_sb[:, off : off + ln],
                )
            nc.sync.dma_start(out=z_d[:], in_=z_sb[:])

    nc.compile()
    _BUILD_CACHE[key] = nc
    return nc


def kernel(x, wg, w1, b1, w2, b2, k):
    from concourse.bass_utils import run_bass_kernel_spmd

    x = np.asarray(x)
    wg = np.asarray(wg)
    w1 = np.asarray(w1)
    b1 = np.asarray(b1)
    w2 = np.asarray(w2)
    b2 = np.asarray(b2)
    k = int(k)

    B, S, D = x.shape
    E = wg.shape[1]
    H = w1.shape[2]
    T = B * S
    assert E == N_CORES, f"expert-parallel layout assumes E == 8, got {E}"

    xf = np.ascontiguousarray(x.reshape(T, D), dtype=np.float32)

    # --- gate + top-k routing (host; needed to build the dispatch shards) ---
    logits = xf @ wg.astype(np.float32)
    logits -= logits.max(axis=1, keepdims=True)
    np.exp(logits, out=logits)
    scores = logits / logits.sum(axis=1, keepdims=True)
    if k >= E:
        topi = np.broadcast_to(np.arange(E, dtype=np.int64), (T, E))
    else:
        topi = np.argpartition(-scores, k, axis=1)[:, :k]
    rows = np.arange(T)[:, None]
    topv = scores[rows, topi]

    # per-expert token lists
    idx_e = []
    val_e = []
    for e in range(E):
        tmask, kpos = np.nonzero(topi == e)
        idx_e.append(tmask)
        val_e.append(topv[tmask, kpos].astype(np.float32))
    max_cnt = max(len(i) for i in idx_e)
    C = max(512, _round_up(max_cnt, 256))

    nc = _build_program(D, H, C, w2.shape[2])

    in_maps = []
    for e in range(E):
        n_e = len(idx_e[e])
        xtg = np.zeros((D, C), dtype=np.float32)
        xtg[:, :n_e] = xf[idx_e[e]].T
        g = np.zeros((1, C), dtype=np.float32)
        g[0, :n_e] = val_e[e]
        b1t = np.ascontiguousarray(
            b1[e].astype(np.float32).reshape(H // P, P).T
        )
        in_maps.append(
            {
                "xtg": xtg,
                "w1": np.ascontiguousarray(w1[e], dtype=np.float32),
                "b1t": b1t,
                "w2": np.ascontiguousarray(w2[e], dtype=np.float32),
                "b2": np.ascontiguousarray(b2[e][None, :], dtype=np.float32),
                "g": g,
            }
        )

    res = run_bass_kernel_spmd(nc, in_maps, core_ids=list(range(N_CORES)))

    # --- combine: scatter-add per-(token, expert) scalars, then log_softmax ---
    s = np.zeros(T, dtype=np.float32)
    for e in range(E):
        n_e = len(idx_e[e])
        if n_e:
            s[idx_e[e]] += res.results[e]["z"][0, :n_e]

    sm = s.reshape(B, S)
    sm = sm - sm.max(axis=1, keepdims=True)
    out = sm - np.log(np.exp(sm).sum(axis=1, keepdims=True))
    return out.astype(np.float32)


# revision 17
# speedup vs baseline: 1.0700x; 1.0700x over previous
"""Expert-parallel MoE routing kernel for Trainium2 (8 NeuronCores).

Problem: top-k(=2) softmax-gated MoE FFN (relu), followed by
log_softmax(sum(moe_out, axis=-1)) over the sequence dim.

Key algebraic observation: the graded output is
    log_softmax_S( sum_d moe_out[t, d] )
and
    sum_d moe_out[t, :] = sum_e combine[t,e] * (relu(x_t @ W1_e + b1_e) @ rowsum(W2_e) + sum(b2_e))
so the second expert matmul collapses to a matvec against rowsum(W2_e).
All of W2 must still be read from HBM (memory-regime roofline unchanged);
its row-sum is computed on-device by the Vector engine while W1 streams
into the Tensor engine.

Sharding (per the expert-parallel hint): core e owns expert e's weights.
The host computes the (tiny) gate/top-k routing to build the dispatch
(it must, to construct the per-core input shards), gathers each expert's
tokens, and the device does the entire FFN including gate-value scaling.
Host then scatter-adds the per-(token,expert) scalars and applies the
final log_softmax on the [B, S] result.

Matmuls run as float32r (fp32 storage, reduced-precision PE mode, 4x the
throughput of strict fp32; measured rel-err ~2e-4 end to end).
"""

import os

import numpy as np

N_CORES = 8
P = 128


def _round_up(v, m):
    return ((v + m - 1) // m) * m


def _chunks(C):
    if C <= 512:
        return [(0, C)]
    if MM_MODE == "bf16":
        # big matmuls amortize per-instruction + weight-load overhead
        out = []
        off = 0
        while off < C:
            ln = min(512, C - off)
            out.append((off, ln))
            off += ln
        return out
    # float32r needs each chunk >=256 to run at its fast rate
    assert C % 256 == 0
    h = C // 2
    return [(0, h), (h, h)]


_BUILD_CACHE = {}

# matmul operand dtype: "bf16" (1 cy/row, in-flight cast on DMA, ~3e-3 rel err)
# or "f32r" (fp32 storage, ~2.8 cy/row, ~1e-4 rel err)
MM_MODE = os.environ.get("MOE_MM_MODE", "bf16")


def _build_program(D, H, C, n_b2):
    """Trace + compile the single-core program (SPMD across 8 cores).

    Per-core inputs:
      xtg [D, C]  f32r  gathered tokens for this expert, transposed
      w1  [D, H]  f32r  expert's first-layer weight (natural = lhsT layout)
      b1t [P, H/P] f32  expert's first-layer bias, column m = b1[m*128:(m+1)*128]
      w2  [H, n_b2] f32 expert's second-layer weight
      b2  [1, n_b2] f32 expert's second-layer bias
      g   [1, C]  f32   gate values per slot (0 for padding slots)
    Output:
      z [1, C] f32 = g * (relu(x @ w1 + b1) @ rowsum(w2) + sum(b2))
    """
    key = (D, H, C, n_b2, MM_MODE)
    if key in _BUILD_CACHE:
        return _BUILD_CACHE[key]

    import concourse.tile as tile
    from concourse import bacc, mybir

    f32 = mybir.dt.float32
    mmdt = mybir.dt.bfloat16 if MM_MODE == "bf16" else mybir.dt.float32r
    KD = D // P  # k-tiles over D
    MH = H // P  # m-tiles over H
    chunks = _chunks(C)

    nc = bacc.Bacc("TRN2", target_bir_lowering=False, debug=False)
    in_dt = f32 if MM_MODE == "bf16" else mybir.dt.float32r
    xtg_d = nc.dram_tensor("xtg", [D, C], in_dt, kind="ExternalInput").ap()
    w1_d = nc.dram_tensor("w1", [D, H], in_dt, kind="ExternalInput").ap()
    b1t_d = nc.dram_tensor("b1t", [P, MH], f32, kind="ExternalInput").ap()
    w2_d = nc.dram_tensor("w2", [H, n_b2], f32, kind="ExternalInput").ap()
    b2_d = nc.dram_tensor("b2", [1, n_b2], f32, kind="ExternalInput").ap()
    g_d = nc.dram_tensor("g", [1, C], f32, kind="ExternalInput").ap()
    z_d = nc.dram_tensor("z", [1, C], f32, kind="ExternalOutput").ap()

    cast_needed = MM_MODE == "bf16"

    with tile.TileContext(nc) as tc:
        with (
            tc.tile_pool(name="persist", bufs=1) as persist,
            tc.tile_pool(name="stage", bufs=4) as stage,
            tc.tile_pool(name="w2p", bufs=3) as w2p,
            tc.tile_pool(name="psum_h", bufs=6, space="PSUM") as psum_h,
            tc.tile_pool(name="psum_z", bufs=2, space="PSUM") as psum_z,
        ):
            def load_cast(dst_ap, src_ap, stage_shape, eng=None):
                # fast HWDGE fp32 load + engine cast copy; plain HWDGE load
                # when no cast is needed
                if cast_needed:
                    st = stage.tile(stage_shape, f32, tag="stage", name="stage")
                    nc.sync.dma_start(out=st[:], in_=src_ap)
                    (eng or nc.vector).tensor_copy(out=dst_ap, in_=st[:])
                else:
                    nc.sync.dma_start(out=dst_ap, in_=src_ap)

            # --- activations, transposed+gathered: 8 tiles [128, C] ---
            # issued first: these + w1 gate the Tensor engine's start
            xtg_tiles = []
            for kd in range(KD):
                t = persist.tile([P, C], mmdt, tag=f"xtg{kd}", name=f"xtg{kd}")
                load_cast(t[:], xtg_d[kd * P : (kd + 1) * P, :], [P, C])
                xtg_tiles.append(t)

            # --- w1: resident k-major tiles, loaded in 512-column pieces
            # (= one m-group's worth) so the Tensor engine starts after the
            # first piece instead of the whole 8 MB; casts run on the
            # otherwise-idle GpSimd engine ---
            GRP = 4
            PIECE = GRP * P
            w1_tiles = []
            for kd in range(KD):
                t = persist.tile([P, H], mmdt, tag=f"w1k{kd}", name=f"w1k{kd}")
                w1_tiles.append(t)
            for p0 in range(0, H, PIECE):
                for kd in range(KD):
                    sl = slice(p0, p0 + PIECE)
                    load_cast(
                        w1_tiles[kd][:, sl],
                        w1_d[kd * P : (kd + 1) * P, sl],
                        [P, PIECE],
                    )

            # --- small persistent loads ---
            g_sb = persist.tile([1, C], f32)
            nc.sync.dma_start(out=g_sb[:], in_=g_d[:])
            b1t_sb = persist.tile([P, MH], f32)
            nc.sync.dma_start(out=b1t_sb[:], in_=b1t_d[:])
            b2_sb = persist.tile([1, n_b2], f32)
            nc.sync.dma_start(out=b2_sb[:], in_=b2_d[:])
            b2sum = persist.tile([1, 1], f32)
            nc.vector.reduce_sum(out=b2sum[:], in_=b2_sb[:], axis=mybir.AxisListType.X)

            # w2 row-sums, column m = rowsum over free dim of w2 m-tile
            w2sum = persist.tile([P, MH], mmdt)
            # hT per chunk: [128, MH * chunk_len], slice m holds h^T m-tile
            ht_tiles = [
                persist.tile([P, MH * ln], mmdt, tag=f"ht{ci}", name=f"ht{ci}")
                for ci, (off, ln) in enumerate(chunks)
            ]

            # --- mm1 + relu: h^T[m-tile] = relu(w1^T x^T + b1) ---
            # m-groups of GRP (aligned to the w1 DMA pieces) with the kd
            # accumulation loop outer-per-group: GRP open PSUM accumulators,
            # so the Tensor engine starts on group 0 right after xtg and the
            # first w1 piece land, and both C-chunks run while the group's
            # weights are resident
            for m0 in range(0, MH, GRP):
                grp = range(m0, min(m0 + GRP, MH))
                for ci, (off, ln) in enumerate(chunks):
                    pss = {}
                    for m in grp:
                        pss[m] = psum_h.tile([P, ln], f32, tag="psh", name="psh")
                    for kd in range(KD):
                        for m in grp:
                            nc.tensor.matmul(
                                pss[m][:],
                                w1_tiles[kd][:, m * P : (m + 1) * P],
                                xtg_tiles[kd][:, off : off + ln],
                                start=(kd == 0),
                                stop=(kd == KD - 1),
                            )
                    for m in grp:
                        nc.scalar.activation(
                            ht_tiles[ci][:, m * ln : (m + 1) * ln],
                            pss[m][:],
                            mybir.ActivationFunctionType.Relu,
                            bias=b1t_sb[:, m : m + 1],
                        )

            # --- w2 stream (DVE row-sums, overlaps the PE loop) ---
            # issued on the sync queue AFTER w1/xtg so it does not steal
            # early HBM bandwidth; w2sum is only needed by the matvec at
            # the very end
            for m in range(MH):
                w2t = w2p.tile([P, n_b2], f32)
                nc.sync.dma_start(out=w2t[:], in_=w2_d[m * P : (m + 1) * P, :])
                with nc.allow_low_precision(
                    reason="w2 row-sum feeds reduced-precision matmul anyway"
                ):
                    nc.vector.reduce_sum(
                        out=w2sum[:, m : m + 1], in_=w2t[:], axis=mybir.AxisListType.X
                    )

            # --- matvec against w2 row-sums + bias + gate scale ---
            z_sb = persist.tile([1, C], f32)
            for ci, (off, ln) in enumerate(chunks):
                pz = psum_z.tile([1, ln], f32)
                for m in range(MH):
                    nc.tensor.matmul(
                        pz[:],
                        w2sum[:, m : m + 1],
                        ht_tiles[ci][:, m * ln : (m + 1) * ln],
                        start=(m == 0),
                        stop=(m == MH - 1),
                    )
                nc.scalar.activation(
                    z_sb[:, off : off + ln],
                    pz[:],
                    mybir.ActivationFunctionType.Identity,
                    bias=b2sum[:],
                )
                nc.vector.tensor_mul(
                    z_sb[:, off : off + ln],
                    z_sb[:, off : off + ln],
                    g_sb[:, off : off + ln],
                )
            nc.sync.dma_start(out=z_d[:], in_=z_sb[:])

    nc.compile()
    _BUILD_CACHE[key] = nc
    return nc


def kernel(x, wg, w1, b1, w2, b2, k):
    from concourse.bass_utils import run_bass_kernel_spmd

    x = np.asarray(x)
    wg = np.asarray(wg)
    w1 = np.asarray(w1)
    b1 = np.asarray(b1)
    w2 = np.asarray(w2)
    b2 = np.asarray(b2)
    k = int(k)

    B, S, D = x.shape
    E = wg.shape[1]
    H = w1.shape[2]
    T = B * S
    assert E == N_CORES, f"expert-parallel layout assumes E == 8, got {E}"

    xf = np.ascontiguousarray(x.reshape(T, D), dtype=np.float32)

    # --- gate + top-k routing (host; needed to build the dispatch shards) ---
    logits = xf @ wg.astype(np.float32)
    logits -= logits.max(axis=1, keepdims=True)
    np.exp(logits, out=logits)
    scores = logits / logits.sum(axis=1, keepdims=True)
    if k >= E:
        topi = np.broadcast_to(np.arange(E, dtype=np.int64), (T, E))
    else:
        topi = np.argpartition(-scores, k, axis=1)[:, :k]
    rows = np.arange(T)[:, None]
    topv = scores[rows, topi]

    # per-expert token lists
    idx_e = []
    val_e = []
    for e in range(E):
        tmask, kpos = np.nonzero(topi == e)
        idx_e.append(tmask)
        val_e.append(topv[tmask, kpos].astype(np.float32))
    max_cnt = max(len(i) for i in idx_e)
    C = max(512, _round_up(max_cnt, 256))

    nc = _build_program(D, H, C, w2.shape[2])

    in_maps = []
    for e in range(E):
        n_e = len(idx_e[e])
        xtg = np.zeros((D, C), dtype=np.float32)
        xtg[:, :n_e] = xf[idx_e[e]].T
        g = np.zeros((1, C), dtype=np.float32)
        g[0, :n_e] = val_e[e]
        b1t = np.ascontiguousarray(
            b1[e].astype(np.float32).reshape(H // P, P).T
        )
        in_maps.append(
            {
                "xtg": xtg,
                "w1": np.ascontiguousarray(w1[e], dtype=np.float32),
                "b1t": b1t,
                "w2": np.ascontiguousarray(w2[e], dtype=np.float32),
                "b2": np.ascontiguousarray(b2[e][None, :], dtype=np.float32),
                "g": g,
            }
        )

    res = run_bass_kernel_spmd(nc, in_maps, core_ids=list(range(N_CORES)))

    # --- combine: scatter-add per-(token, expert) scalars, then log_softmax ---
    s = np.zeros(T, dtype=np.float32)
    for e in range(E):
        n_e = len(idx_e[e])
        if n_e:
            s[idx_e[e]] += res.results[e]["z"][0, :n_e]

    sm = s.reshape(B, S)
    sm = sm - sm.max(axis=1, keepdims=True)
    out = sm - np.log(np.exp(sm).sum(axis=1, keepdims=True))
    return out.astype(np.float32)


# revision 18
# speedup vs baseline: 1.3111x; 1.2253x over previous
"""Expert-parallel MoE routing kernel for Trainium2 (8 NeuronCores).

Problem: top-k(=2) softmax-gated MoE FFN (relu), followed by
log_softmax(sum(moe_out, axis=-1)) over the sequence dim.

Key algebraic observation: the graded output is
    log_softmax_S( sum_d moe_out[t, d] )
and
    sum_d moe_out[t, :] = sum_e combine[t,e] * (relu(x_t @ W1_e + b1_e) @ rowsum(W2_e) + sum(b2_e))
so the second expert matmul collapses to a matvec against rowsum(W2_e).
All of W2 must still be read from HBM (memory-regime roofline unchanged);
its row-sum is computed on-device by the Vector engine while W1 streams
into the Tensor engine.

Sharding (per the expert-parallel hint): core e owns expert e's weights.
The host computes the (tiny) gate/top-k routing to build the dispatch
(it must, to construct the per-core input shards), gathers each expert's
tokens, and the device does the entire FFN including gate-value scaling.
Host then scatter-adds the per-(token,expert) scalars and applies the
final log_softmax on the [B, S] result.

Matmuls run as float32r (fp32 storage, reduced-precision PE mode, 4x the
throughput of strict fp32; measured rel-err ~2e-4 end to end).
"""

import os

import numpy as np

N_CORES = 8
P = 128


def _round_up(v, m):
    return ((v + m - 1) // m) * m


def _chunks(C):
    if C <= 512:
        return [(0, C)]
    if MM_MODE == "bf16":
        # big matmuls amortize per-instruction + weight-load overhead
        out = []
        off = 0
        while off < C:
            ln = min(512, C - off)
            out.append((off, ln))
            off += ln
        return out
    # float32r needs each chunk >=256 to run at its fast rate
    assert C % 256 == 0
    h = C // 2
    return [(0, h), (h, h)]


_BUILD_CACHE = {}

# matmul operand dtype: "bf16" (1 cy/row, in-flight cast on DMA, ~3e-3 rel err)
# or "f32r" (fp32 storage, ~2.8 cy/row, ~1e-4 rel err)
MM_MODE = os.environ.get("MOE_MM_MODE", "bf16")


def _build_program(D, H, C, n_b2):
    """Trace + compile the single-core program (SPMD across 8 cores).

    Per-core inputs:
      xtg [D, C]  f32r  gathered tokens for this expert, transposed
      w1  [D, H]  f32r  expert's first-layer weight (natural = lhsT layout)
      b1t [P, H/P] f32  expert's first-layer bias, column m = b1[m*128:(m+1)*128]
      w2  [H, n_b2] f32 expert's second-layer weight
      b2  [1, n_b2] f32 expert's second-layer bias
      g   [1, C]  f32   gate values per slot (0 for padding slots)
    Output:
      z [1, C] f32 = g * (relu(x @ w1 + b1) @ rowsum(w2) + sum(b2))
    """
    key = (D, H, C, n_b2, MM_MODE)
    if key in _BUILD_CACHE:
        return _BUILD_CACHE[key]

    import concourse.tile as tile
    from concourse import bacc, mybir

    f32 = mybir.dt.float32
    mmdt = mybir.dt.bfloat16 if MM_MODE == "bf16" else mybir.dt.float32r
    KD = D // P  # k-tiles over D
    MH = H // P  # m-tiles over H
    chunks = _chunks(C)

    nc = bacc.Bacc("TRN2", target_bir_lowering=False, debug=False)
    in_dt = f32 if MM_MODE == "bf16" else mybir.dt.float32r
    xtg_d = nc.dram_tensor("xtg", [D, C], in_dt, kind="ExternalInput").ap()
    w1_d = nc.dram_tensor("w1", [D, H], in_dt, kind="ExternalInput").ap()
    b1t_d = nc.dram_tensor("b1t", [P, MH], f32, kind="ExternalInput").ap()
    w2_d = nc.dram_tensor("w2", [H, n_b2], f32, kind="ExternalInput").ap()
    b2_d = nc.dram_tensor("b2", [1, n_b2], f32, kind="ExternalInput").ap()
    g_d = nc.dram_tensor("g", [1, C], f32, kind="ExternalInput").ap()
    z_d = nc.dram_tensor("z", [1, C], f32, kind="ExternalOutput").ap()

    cast_needed = MM_MODE == "bf16"

    with tile.TileContext(nc) as tc:
        with (
            tc.tile_pool(name="persist", bufs=1) as persist,
            tc.tile_pool(name="stage", bufs=14) as stage,
            tc.tile_pool(name="w2p", bufs=3) as w2p,
            tc.tile_pool(name="psum_h", bufs=6, space="PSUM") as psum_h,
            tc.tile_pool(name="psum_z", bufs=2, space="PSUM") as psum_z,
        ):
            def load_cast(dst_ap, src_ap, stage_shape, eng=None):
                # fast HWDGE fp32 load + engine cast copy; plain HWDGE load
                # when no cast is needed
                if cast_needed:
                    st = stage.tile(stage_shape, f32, tag="stage", name="stage")
                    nc.sync.dma_start(out=st[:], in_=src_ap)
                    (eng or nc.vector).tensor_copy(out=dst_ap, in_=st[:])
                else:
                    nc.sync.dma_start(out=dst_ap, in_=src_ap)

            # --- activations, transposed+gathered: 8 tiles [128, C] ---
            # issued first: these + w1 gate the Tensor engine's start
            xtg_tiles = []
            for kd in range(KD):
                t = persist.tile([P, C], mmdt, tag=f"xtg{kd}", name=f"xtg{kd}")
                load_cast(t[:], xtg_d[kd * P : (kd + 1) * P, :], [P, C])
                xtg_tiles.append(t)

            # --- w1: resident k-major tiles, loaded in 512-column pieces
            # (= one m-group's worth) so the Tensor engine starts after the
            # first piece instead of the whole 8 MB; casts run on the
            # otherwise-idle GpSimd engine ---
            GRP = 4
            PIECE = GRP * P
            w1_tiles = []
            for kd in range(KD):
                t = persist.tile([P, H], mmdt, tag=f"w1k{kd}", name=f"w1k{kd}")
                w1_tiles.append(t)
            for p0 in range(0, H, PIECE):
                for kd in range(KD):
                    sl = slice(p0, p0 + PIECE)
                    load_cast(
                        w1_tiles[kd][:, sl],
                        w1_d[kd * P : (kd + 1) * P, sl],
                        [P, PIECE],
                    )

            # --- small persistent loads ---
            g_sb = persist.tile([1, C], f32)
            nc.sync.dma_start(out=g_sb[:], in_=g_d[:])
            b1t_sb = persist.tile([P, MH], f32)
            nc.sync.dma_start(out=b1t_sb[:], in_=b1t_d[:])
            b2_sb = persist.tile([1, n_b2], f32)
            nc.sync.dma_start(out=b2_sb[:], in_=b2_d[:])
            b2sum = persist.tile([1, 1], f32)
            nc.vector.reduce_sum(out=b2sum[:], in_=b2_sb[:], axis=mybir.AxisListType.X)

            # w2 row-sums, column m = rowsum over free dim of w2 m-tile
            w2sum = persist.tile([P, MH], mmdt)
            # hT per chunk: [128, MH * chunk_len], slice m holds h^T m-tile
            ht_tiles = [
                persist.tile([P, MH * ln], mmdt, tag=f"ht{ci}", name=f"ht{ci}")
                for ci, (off, ln) in enumerate(chunks)
            ]

            # --- mm1 + relu: h^T[m-tile] = relu(w1^T x^T + b1) ---
            # m-groups of GRP (aligned to the w1 DMA pieces) with the kd
            # accumulation loop outer-per-group: GRP open PSUM accumulators,
            # so the Tensor engine starts on group 0 right after xtg and the
            # first w1 piece land, and both C-chunks run while the group's
            # weights are resident
            for m0 in range(0, MH, GRP):
                grp = range(m0, min(m0 + GRP, MH))
                for ci, (off, ln) in enumerate(chunks):
                    pss = {}
                    for m in grp:
                        pss[m] = psum_h.tile([P, ln], f32, tag="psh", name="psh")
                    for kd in range(KD):
                        for m in grp:
                            nc.tensor.matmul(
                                pss[m][:],
                                w1_tiles[kd][:, m * P : (m + 1) * P],
                                xtg_tiles[kd][:, off : off + ln],
                                start=(kd == 0),
                                stop=(kd == KD - 1),
                            )
                    for m in grp:
                        nc.scalar.activation(
                            ht_tiles[ci][:, m * ln : (m + 1) * ln],
                            pss[m][:],
                            mybir.ActivationFunctionType.Relu,
                            bias=b1t_sb[:, m : m + 1],
                        )

            # --- w2 stream (DVE row-sums, overlaps the PE loop) ---
            # issued on the sync queue AFTER w1/xtg so it does not steal
            # early HBM bandwidth; w2sum is only needed by the matvec at
            # the very end
            for m in range(MH):
                w2t = w2p.tile([P, n_b2], f32)
                nc.sync.dma_start(out=w2t[:], in_=w2_d[m * P : (m + 1) * P, :])
                with nc.allow_low_precision(
                    reason="w2 row-sum feeds reduced-precision matmul anyway"
                ):
                    nc.vector.reduce_sum(
                        out=w2sum[:, m : m + 1], in_=w2t[:], axis=mybir.AxisListType.X
                    )

            # --- matvec against w2 row-sums + bias + gate scale ---
            z_sb = persist.tile([1, C], f32)
            for ci, (off, ln) in enumerate(chunks):
                pz = psum_z.tile([1, ln], f32)
                for m in range(MH):
                    nc.tensor.matmul(
                        pz[:],
                        w2sum[:, m : m + 1],
                        ht_tiles[ci][:, m * ln : (m + 1) * ln],
                        start=(m == 0),
                        stop=(m == MH - 1),
                    )
                nc.scalar.activation(
                    z_sb[:, off : off + ln],
                    pz[:],
                    mybir.ActivationFunctionType.Identity,
                    bias=b2sum[:],
                )
                nc.vector.tensor_mul(
                    z_sb[:, off : off + ln],
                    z_sb[:, off : off + ln],
                    g_sb[:, off : off + ln],
                )
            nc.sync.dma_start(out=z_d[:], in_=z_sb[:])

    nc.compile()
    _BUILD_CACHE[key] = nc
    return nc


def kernel(x, wg, w1, b1, w2, b2, k):
    from concourse.bass_utils import run_bass_kernel_spmd

    x = np.asarray(x)
    wg = np.asarray(wg)
    w1 = np.asarray(w1)
    b1 = np.asarray(b1)
    w2 = np.asarray(w2)
    b2 = np.asarray(b2)
    k = int(k)

    B, S, D = x.shape
    E = wg.shape[1]
    H = w1.shape[2]
    T = B * S
    assert E == N_CORES, f"expert-parallel layout assumes E == 8, got {E}"

    xf = np.ascontiguousarray(x.reshape(T, D), dtype=np.float32)

    # --- gate + top-k routing (host; needed to build the dispatch shards) ---
    logits = xf @ wg.astype(np.float32)
    logits -= logits.max(axis=1, keepdims=True)
    np.exp(logits, out=logits)
    scores = logits / logits.sum(axis=1, keepdims=True)
    if k >= E:
        topi = np.broadcast_to(np.arange(E, dtype=np.int64), (T, E))
    else:
        topi = np.argpartition(-scores, k, axis=1)[:, :k]
    rows = np.arange(T)[:, None]
    topv = scores[rows, topi]

    # per-expert token lists
    idx_e = []
    val_e = []
    for e in range(E):
        tmask, kpos = np.nonzero(topi == e)
        idx_e.append(tmask)
        val_e.append(topv[tmask, kpos].astype(np.float32))
    max_cnt = max(len(i) for i in idx_e)
    C = max(512, _round_up(max_cnt, 256))

    nc = _build_program(D, H, C, w2.shape[2])

    in_maps = []
    for e in range(E):
        n_e = len(idx_e[e])
        xtg = np.zeros((D, C), dtype=np.float32)
        xtg[:, :n_e] = xf[idx_e[e]].T
        g = np.zeros((1, C), dtype=np.float32)
        g[0, :n_e] = val_e[e]
        b1t = np.ascontiguousarray(
            b1[e].astype(np.float32).reshape(H // P, P).T
        )
        in_maps.append(
            {
                "xtg": xtg,
                "w1": np.ascontiguousarray(w1[e], dtype=np.float32),
                "b1t": b1t,
                "w2": np.ascontiguousarray(w2[e], dtype=np.float32),
                "b2": np.ascontiguousarray(b2[e][None, :], dtype=np.float32),
                "g": g,
            }
        )

    res = run_bass_kernel_spmd(nc, in_maps, core_ids=list(range(N_CORES)))

    # --- combine: scatter-add per-(token, expert) scalars, then log_softmax ---
    s = np.zeros(T, dtype=np.float32)
    for e in range(E):
        n_e = len(idx_e[e])
        if n_e:
            s[idx_e[e]] += res.results[e]["z"][0, :n_e]

    sm = s.reshape(B, S)
    sm = sm - sm.max(axis=1, keepdims=True)
    out = sm - np.log(np.exp(sm).sum(axis=1, keepdims=True))
    return out.astype(np.float32)


# revision 22
# speedup vs baseline: 1.3731x; 1.0473x over previous
"""Expert-parallel MoE routing kernel for Trainium2 (8 NeuronCores).

Problem: top-k(=2) softmax-gated MoE FFN (relu), followed by
log_softmax(sum(moe_out, axis=-1)) over the sequence dim.

Key algebraic observation: the graded output is
    log_softmax_S( sum_d moe_out[t, d] )
and
    sum_d moe_out[t, :] = sum_e combine[t,e] * (relu(x_t @ W1_e + b1_e) @ rowsum(W2_e) + sum(b2_e))
so the second expert matmul collapses to a matvec against rowsum(W2_e).
All of W2 must still be read from HBM (memory-regime roofline unchanged);
its row-sum is computed on-device by the Vector engine while W1 streams
into the Tensor engine.

Sharding (per the expert-parallel hint): core e owns expert e's weights.
The host computes the (tiny) gate/top-k routing to build the dispatch
(it must, to construct the per-core input shards), gathers each expert's
tokens, and the device does the entire FFN including gate-value scaling.
Host then scatter-adds the per-(token,expert) scalars and applies the
final log_softmax on the [B, S] result.

Matmuls run as float32r (fp32 storage, reduced-precision PE mode, 4x the
throughput of strict fp32; measured rel-err ~2e-4 end to end).
"""

import os

import numpy as np

N_CORES = 8
P = 128


def _round_up(v, m):
    return ((v + m - 1) // m) * m


def _chunks(C):
    if C <= 512:
        return [(0, C)]
    if MM_MODE == "bf16":
        # big matmuls amortize per-instruction + weight-load overhead
        out = []
        off = 0
        while off < C:
            ln = min(512, C - off)
            out.append((off, ln))
            off += ln
        return out
    # float32r needs each chunk >=256 to run at its fast rate
    assert C % 256 == 0
    h = C // 2
    return [(0, h), (h, h)]


_BUILD_CACHE = {}

# matmul operand dtype: "bf16" (1 cy/row, in-flight cast on DMA, ~3e-3 rel err)
# or "f32r" (fp32 storage, ~2.8 cy/row, ~1e-4 rel err)
MM_MODE = os.environ.get("MOE_MM_MODE", "bf16")


def _build_program(D, H, C, n_b2):
    """Trace + compile the single-core program (SPMD across 8 cores).

    Per-core inputs:
      xtg [D, C]  f32r  gathered tokens for this expert, transposed
      w1  [D, H]  f32r  expert's first-layer weight (natural = lhsT layout)
      b1t [P, H/P] f32  expert's first-layer bias, column m = b1[m*128:(m+1)*128]
      w2  [H, n_b2] f32 expert's second-layer weight
      b2  [1, n_b2] f32 expert's second-layer bias
      g   [1, C]  f32   gate values per slot (0 for padding slots)
    Output:
      z [1, C] f32 = g * (relu(x @ w1 + b1) @ rowsum(w2) + sum(b2))
    """
    key = (D, H, C, n_b2, MM_MODE)
    if key in _BUILD_CACHE:
        return _BUILD_CACHE[key]

    import concourse.tile as tile
    from concourse import bacc, mybir

    f32 = mybir.dt.float32
    mmdt = mybir.dt.bfloat16 if MM_MODE == "bf16" else mybir.dt.float32r
    KD = D // P  # k-tiles over D
    MH = H // P  # m-tiles over H
    chunks = _chunks(C)

    nc = bacc.Bacc("TRN2", target_bir_lowering=False, debug=False)
    in_dt = f32 if MM_MODE == "bf16" else mybir.dt.float32r
    xtg_d = nc.dram_tensor("xtg", [D, C], in_dt, kind="ExternalInput").ap()
    w1_d = nc.dram_tensor("w1", [D, H], in_dt, kind="ExternalInput").ap()
    b1t_d = nc.dram_tensor("b1t", [P, MH], f32, kind="ExternalInput").ap()
    w2_d = nc.dram_tensor("w2", [H, n_b2], f32, kind="ExternalInput").ap()
    b2_d = nc.dram_tensor("b2", [1, n_b2], f32, kind="ExternalInput").ap()
    g_d = nc.dram_tensor("g", [1, C], f32, kind="ExternalInput").ap()
    z_d = nc.dram_tensor("z", [1, C], f32, kind="ExternalOutput").ap()

    cast_needed = MM_MODE == "bf16"

    with tile.TileContext(nc) as tc:
        with (
            tc.tile_pool(name="persist", bufs=1) as persist,
            tc.tile_pool(name="stage", bufs=14) as stage,
            tc.tile_pool(name="w2p", bufs=3) as w2p,
            tc.tile_pool(name="psum_h", bufs=6, space="PSUM") as psum_h,
            tc.tile_pool(name="psum_z", bufs=2, space="PSUM") as psum_z,
        ):
            def load_cast(dst_ap, src_ap, stage_shape, eng=None):
                # fast HWDGE fp32 load + engine cast copy; plain HWDGE load
                # when no cast is needed
                if cast_needed:
                    st = stage.tile(stage_shape, f32, tag="stage", name="stage")
                    nc.sync.dma_start(out=st[:], in_=src_ap)
                    if eng is nc.scalar:
                        nc.scalar.copy(out=dst_ap, in_=st[:])
                    else:
                        (eng or nc.vector).tensor_copy(out=dst_ap, in_=st[:])
                else:
                    nc.sync.dma_start(out=dst_ap, in_=src_ap)

            # --- small loads first: a few KB, and the relu bias (b1t) gates
            # PSUM recycling so it must not queue behind the weight stream ---
            g_sb = persist.tile([1, C], f32)
            nc.sync.dma_start(out=g_sb[:], in_=g_d[:])
            b1t_sb = persist.tile([P, MH], f32)
            nc.sync.dma_start(out=b1t_sb[:], in_=b1t_d[:])
            b2_sb = persist.tile([1, n_b2], f32)
            nc.sync.dma_start(out=b2_sb[:], in_=b2_d[:])

            # --- activations, transposed+gathered: 8 tiles [128, C] ---
            # issued first: these + w1 gate the Tensor engine's start
            xtg_tiles = []
            for kd in range(KD):
                t = persist.tile([P, C], mmdt, tag=f"xtg{kd}", name=f"xtg{kd}")
                load_cast(
                    t[:], xtg_d[kd * P : (kd + 1) * P, :], [P, C], eng=nc.scalar
                )
                xtg_tiles.append(t)

            # --- w1: resident k-major tiles, loaded in 512-column pieces
            # (= one m-group's worth) so the Tensor engine starts after the
            # first piece instead of the whole 8 MB; casts run on the
            # otherwise-idle GpSimd engine ---
            GRP = 4
            PIECE = GRP * P
            w1_tiles = []
            for kd in range(KD):
                t = persist.tile([P, H], mmdt, tag=f"w1k{kd}", name=f"w1k{kd}")
                w1_tiles.append(t)
            for p0 in range(0, H, PIECE):
                for kd in range(KD):
                    sl = slice(p0, p0 + PIECE)
                    load_cast(
                        w1_tiles[kd][:, sl],
                        w1_d[kd * P : (kd + 1) * P, sl],
                        [P, PIECE],
                    )

            b2sum = persist.tile([1, 1], f32)
            nc.vector.reduce_sum(out=b2sum[:], in_=b2_sb[:], axis=mybir.AxisListType.X)

            # w2 row-sums, column m = rowsum over free dim of w2 m-tile
            w2sum = persist.tile([P, MH], mmdt)
            # hT per chunk: [128, MH * chunk_len], slice m holds h^T m-tile
            ht_tiles = [
                persist.tile([P, MH * ln], mmdt, tag=f"ht{ci}", name=f"ht{ci}")
                for ci, (off, ln) in enumerate(chunks)
            ]

            # --- mm1 + relu: h^T[m-tile] = relu(w1^T x^T + b1) ---
            # m-groups of GRP (aligned to the w1 DMA pieces) with the kd
            # accumulation loop outer-per-group: GRP open PSUM accumulators,
            # so the Tensor engine starts on group 0 right after xtg and the
            # first w1 piece land, and both C-chunks run while the group's
            # weights are resident
            for m0 in range(0, MH, GRP):
                grp = range(m0, min(m0 + GRP, MH))
                for ci, (off, ln) in enumerate(chunks):
                    pss = {}
                    for m in grp:
                        pss[m] = psum_h.tile([P, ln], f32, tag="psh", name="psh")
                    for kd in range(KD):
                        for m in grp:
                            nc.tensor.matmul(
                                pss[m][:],
                                w1_tiles[kd][:, m * P : (m + 1) * P],
                                xtg_tiles[kd][:, off : off + ln],
                                start=(kd == 0),
                                stop=(kd == KD - 1),
                            )
                    for m in grp:
                        nc.scalar.activation(
                            ht_tiles[ci][:, m * ln : (m + 1) * ln],
                            pss[m][:],
                            mybir.ActivationFunctionType.Relu,
                            bias=b1t_sb[:, m : m + 1],
                        )

            # --- w2 stream (DVE row-sums, overlaps the PE loop) ---
            # issued on the sync queue AFTER w1/xtg so it does not steal
            # early HBM bandwidth; w2sum is only needed by the matvec at
            # the very end
            for m in range(MH):
                w2t = w2p.tile([P, n_b2], f32)
                nc.sync.dma_start(out=w2t[:], in_=w2_d[m * P : (m + 1) * P, :])
                with nc.allow_low_precision(
                    reason="w2 row-sum feeds reduced-precision matmul anyway"
                ):
                    nc.vector.reduce_sum(
                        out=w2sum[:, m : m + 1], in_=w2t[:], axis=mybir.AxisListType.X
                    )

            # --- matvec against w2 row-sums + bias + gate scale ---
            z_sb = persist.tile([1, C], f32)
            for ci, (off, ln) in enumerate(chunks):
                pz = psum_z.tile([1, ln], f32)
                for m in range(MH):
                    nc.tensor.matmul(
                        pz[:],
                        w2sum[:, m : m + 1],
                        ht_tiles[ci][:, m * ln : (m + 1) * ln],
                        start=(m == 0),
                        stop=(m == MH - 1),
                    )
                nc.scalar.activation(
                    z_sb[:, off : off + ln],
                    pz[:],
                    mybir.ActivationFunctionType.Identity,
                    bias=b2sum[:],
                )
                nc.vector.tensor_mul(
                    z_sb[:, off : off + ln],
                    z_sb[:, off : off + ln],
                    g_sb[:, off : off + ln],
                )
            nc.sync.dma_start(out=z_d[:], in_=z_sb[:])

    nc.compile()
    _BUILD_CACHE[key] = nc
    return nc


def kernel(x, wg, w1, b1, w2, b2, k):
    from concourse.bass_utils import run_bass_kernel_spmd

    x = np.asarray(x)
    wg = np.asarray(wg)
    w1 = np.asarray(w1)
    b1 = np.asarray(b1)
    w2 = np.asarray(w2)
    b2 = np.asarray(b2)
    k = int(k)

    B, S, D = x.shape
    E = wg.shape[1]
    H = w1.shape[2]
    T = B * S
    assert E == N_CORES, f"expert-parallel layout assumes E == 8, got {E}"

    xf = np.ascontiguousarray(x.reshape(T, D), dtype=np.float32)

    # --- gate + top-k routing (host; needed to build the dispatch shards) ---
    logits = xf @ wg.astype(np.float32)
    logits -= logits.max(axis=1, keepdims=True)
    np.exp(logits, out=logits)
    scores = logits / logits.sum(axis=1, keepdims=True)
    if k >= E:
        topi = np.broadcast_to(np.arange(E, dtype=np.int64), (T, E))
    else:
        topi = np.argpartition(-scores, k, axis=1)[:, :k]
    rows = np.arange(T)[:, None]
    topv = scores[rows, topi]

    # per-expert token lists
    idx_e = []
    val_e = []
    for e in range(E):
        tmask, kpos = np.nonzero(topi == e)
        idx_e.append(tmask)
        val_e.append(topv[tmask, kpos].astype(np.float32))
    max_cnt = max(len(i) for i in idx_e)
    C = max(512, _round_up(max_cnt, 256))

    nc = _build_program(D, H, C, w2.shape[2])

    in_maps = []
    for e in range(E):
        n_e = len(idx_e[e])
        xtg = np.zeros((D, C), dtype=np.float32)
        xtg[:, :n_e] = xf[idx_e[e]].T
        g = np.zeros((1, C), dtype=np.float32)
        g[0, :n_e] = val_e[e]
        b1t = np.ascontiguousarray(
            b1[e].astype(np.float32).reshape(H // P, P).T
        )
        in_maps.append(
            {
                "xtg": xtg,
                "w1": np.ascontiguousarray(w1[e], dtype=np.float32),
                "b1t": b1t,
                "w2": np.ascontiguousarray(w2[e], dtype=np.float32),
                "b2": np.ascontiguousarray(b2[e][None, :], dtype=np.float32),
                "g": g,
            }
        )

    res = run_bass_kernel_spmd(nc, in_maps, core_ids=list(range(N_CORES)))

    # --- combine: scatter-add per-(token, expert) scalars, then log_softmax ---
    s = np.zeros(T, dtype=np.float32)
    for e in range(E):
        n_e = len(idx_e[e])
        if n_e:
            s[idx_e[e]] += res.results[e]["z"][0, :n_e]

    sm = s.reshape(B, S)
    sm = sm - sm.max(axis=1, keepdims=True)
    out = sm - np.log(np.exp(sm).sum(axis=1, keepdims=True))
    return out.astype(np.float32)


# revision 25
# speedup vs baseline: 1.3918x; 1.0137x over previous
"""Expert-parallel MoE routing kernel for Trainium2 (8 NeuronCores).

Problem: top-k(=2) softmax-gated MoE FFN (relu), followed by
log_softmax(sum(moe_out, axis=-1)) over the sequence dim.

Key algebraic observation: the graded output is
    log_softmax_S( sum_d moe_out[t, d] )
and
    sum_d moe_out[t, :] = sum_e combine[t,e] * (relu(x_t @ W1_e + b1_e) @ rowsum(W2_e) + sum(b2_e))
so the second expert matmul collapses to a matvec against rowsum(W2_e).
All of W2 must still be read from HBM (memory-regime roofline unchanged);
its row-sum is computed on-device by the Vector engine while W1 streams
into the Tensor engine.

Sharding (per the expert-parallel hint): core e owns expert e's weights.
The host computes the (tiny) gate/top-k routing to build the dispatch
(it must, to construct the per-core input shards), gathers each expert's
tokens, and the device does the entire FFN including gate-value scaling.
Host then scatter-adds the per-(token,expert) scalars and applies the
final log_softmax on the [B, S] result.

Matmuls run as float32r (fp32 storage, reduced-precision PE mode, 4x the
throughput of strict fp32; measured rel-err ~2e-4 end to end).
"""

import os

import numpy as np

N_CORES = 8
P = 128


def _round_up(v, m):
    return ((v + m - 1) // m) * m


def _chunks(C):
    # even halves keep every matmul's streaming time >= the ~100ns weight
    # load, so LDWEIGHTS stays hidden behind the previous matmul (and f32r
    # needs each chunk >= 256 for its fast rate)
    if C <= 512:
        return [(0, C)]
    assert C % 256 == 0
    h = C // 2
    return [(0, h), (h, h)]


_BUILD_CACHE = {}

# matmul operand dtype: "bf16" (1 cy/row, in-flight cast on DMA, ~3e-3 rel err)
# or "f32r" (fp32 storage, ~2.8 cy/row, ~1e-4 rel err)
MM_MODE = os.environ.get("MOE_MM_MODE", "bf16")


def _build_program(D, H, C, n_b2):
    """Trace + compile the single-core program (SPMD across 8 cores).

    Per-core inputs:
      xtg [D, C]  f32r  gathered tokens for this expert, transposed
      w1  [D, H]  f32r  expert's first-layer weight (natural = lhsT layout)
      b1t [P, H/P] f32  expert's first-layer bias, column m = b1[m*128:(m+1)*128]
      w2  [H, n_b2] f32 expert's second-layer weight
      b2  [1, n_b2] f32 expert's second-layer bias
      g   [1, C]  f32   gate values per slot (0 for padding slots)
    Output:
      z [1, C] f32 = g * (relu(x @ w1 + b1) @ rowsum(w2) + sum(b2))
    """
    key = (D, H, C, n_b2, MM_MODE)
    if key in _BUILD_CACHE:
        return _BUILD_CACHE[key]

    import concourse.tile as tile
    from concourse import bacc, mybir

    f32 = mybir.dt.float32
    mmdt = mybir.dt.bfloat16 if MM_MODE == "bf16" else mybir.dt.float32r
    KD = D // P  # k-tiles over D
    MH = H // P  # m-tiles over H
    chunks = _chunks(C)

    nc = bacc.Bacc("TRN2", target_bir_lowering=False, debug=False)
    in_dt = f32 if MM_MODE == "bf16" else mybir.dt.float32r
    xtg_d = nc.dram_tensor("xtg", [D, C], in_dt, kind="ExternalInput").ap()
    w1_d = nc.dram_tensor("w1", [D, H], in_dt, kind="ExternalInput").ap()
    b1t_d = nc.dram_tensor("b1t", [P, MH], f32, kind="ExternalInput").ap()
    w2_d = nc.dram_tensor("w2", [H, n_b2], f32, kind="ExternalInput").ap()
    b2_d = nc.dram_tensor("b2", [1, n_b2], f32, kind="ExternalInput").ap()
    g_d = nc.dram_tensor("g", [1, C], f32, kind="ExternalInput").ap()
    z_d = nc.dram_tensor("z", [1, C], f32, kind="ExternalOutput").ap()

    cast_needed = MM_MODE == "bf16"

    with tile.TileContext(nc) as tc:
        with (
            tc.tile_pool(name="persist", bufs=1) as persist,
            tc.tile_pool(name="stage", bufs=14) as stage,
            tc.tile_pool(name="w2p", bufs=3) as w2p,
            tc.tile_pool(name="psum_h", bufs=6, space="PSUM") as psum_h,
            tc.tile_pool(name="psum_z", bufs=2, space="PSUM") as psum_z,
        ):
            def load_cast(dst_ap, src_ap, stage_shape, eng=None, tag="stage"):
                # fast HWDGE fp32 load + engine cast copy; plain HWDGE load
                # when no cast is needed
                if cast_needed:
                    st = stage.tile(stage_shape, f32, tag=tag, name=tag)
                    nc.sync.dma_start(out=st[:], in_=src_ap)
                    if eng is nc.scalar:
                        nc.scalar.copy(out=dst_ap, in_=st[:])
                    else:
                        (eng or nc.vector).tensor_copy(out=dst_ap, in_=st[:])
                else:
                    nc.sync.dma_start(out=dst_ap, in_=src_ap)

            # --- small loads first: a few KB, and the relu bias (b1t) gates
            # PSUM recycling so it must not queue behind the weight stream ---
            g_sb = persist.tile([1, C], f32)
            nc.sync.dma_start(out=g_sb[:], in_=g_d[:])
            b1t_sb = persist.tile([P, MH], f32)
            nc.sync.dma_start(out=b1t_sb[:], in_=b1t_d[:])
            b2_sb = persist.tile([1, n_b2], f32)
            nc.sync.dma_start(out=b2_sb[:], in_=b2_d[:])

            # --- activations, transposed+gathered: 8 tiles [128, C] ---
            # issued first: these + w1 gate the Tensor engine's start
            # casts alternate DVE/ACT so the 8 of them run in parallel pairs,
            # on a dedicated stage tag so they don't hold up w1 staging
            xtg_tiles = []
            for kd in range(KD):
                t = persist.tile([P, C], mmdt, tag=f"xtg{kd}", name=f"xtg{kd}")
                load_cast(
                    t[:],
                    xtg_d[kd * P : (kd + 1) * P, :],
                    [P, C],
                    eng=nc.scalar if kd % 2 else nc.vector,
                    tag="xstage",
                )
                xtg_tiles.append(t)

            # --- w1: resident k-major tiles, loaded in 512-column pieces
            # (= one m-group's worth) so the Tensor engine starts after the
            # first piece instead of the whole 8 MB; casts run on the
            # otherwise-idle GpSimd engine ---
            GRP = 4
            PIECE = GRP * P
            w1_tiles = []
            for kd in range(KD):
                t = persist.tile([P, H], mmdt, tag=f"w1k{kd}", name=f"w1k{kd}")
                w1_tiles.append(t)
            for p0 in range(0, H, PIECE):
                for kd in range(KD):
                    sl = slice(p0, p0 + PIECE)
                    load_cast(
                        w1_tiles[kd][:, sl],
                        w1_d[kd * P : (kd + 1) * P, sl],
                        [P, PIECE],
                    )

            b2sum = persist.tile([1, 1], f32)
            nc.vector.reduce_sum(out=b2sum[:], in_=b2_sb[:], axis=mybir.AxisListType.X)

            # w2 row-sums, column m = rowsum over free dim of w2 m-tile
            w2sum = persist.tile([P, MH], mmdt)
            # hT per chunk: [128, MH * chunk_len], slice m holds h^T m-tile
            ht_tiles = [
                persist.tile([P, MH * ln], mmdt, tag=f"ht{ci}", name=f"ht{ci}")
                for ci, (off, ln) in enumerate(chunks)
            ]

            # --- mm1 + relu: h^T[m-tile] = relu(w1^T x^T + b1) ---
            # m-groups of GRP (aligned to the w1 DMA pieces) with the kd
            # accumulation loop outer-per-group: GRP open PSUM accumulators,
            # so the Tensor engine starts on group 0 right after xtg and the
            # first w1 piece land, and both C-chunks run while the group's
            # weights are resident
            for m0 in range(0, MH, GRP):
                grp = range(m0, min(m0 + GRP, MH))
                for ci, (off, ln) in enumerate(chunks):
                    pss = {}
                    for m in grp:
                        pss[m] = psum_h.tile([P, ln], f32, tag="psh", name="psh")
                    for kd in range(KD):
                        for m in grp:
                            nc.tensor.matmul(
                                pss[m][:],
                                w1_tiles[kd][:, m * P : (m + 1) * P],
                                xtg_tiles[kd][:, off : off + ln],
                                start=(kd == 0),
                                stop=(kd == KD - 1),
                            )
                    for m in grp:
                        nc.scalar.activation(
                            ht_tiles[ci][:, m * ln : (m + 1) * ln],
                            pss[m][:],
                            mybir.ActivationFunctionType.Relu,
                            bias=b1t_sb[:, m : m + 1],
                        )

            # --- w2 stream (DVE row-sums, overlaps the PE loop) ---
            # issued on the sync queue AFTER w1/xtg so it does not steal
            # early HBM bandwidth; w2sum is only needed by the matvec at
            # the very end
            for m in range(MH):
                w2t = w2p.tile([P, n_b2], f32)
                nc.sync.dma_start(out=w2t[:], in_=w2_d[m * P : (m + 1) * P, :])
                with nc.allow_low_precision(
                    reason="w2 row-sum feeds reduced-precision matmul anyway"
                ):
                    nc.vector.reduce_sum(
                        out=w2sum[:, m : m + 1], in_=w2t[:], axis=mybir.AxisListType.X
                    )

            # --- matvec against w2 row-sums + bias + gate scale ---
            z_sb = persist.tile([1, C], f32)
            for ci, (off, ln) in enumerate(chunks):
                pz = psum_z.tile([1, ln], f32)
                for m in range(MH):
                    nc.tensor.matmul(
                        pz[:],
                        w2sum[:, m : m + 1],
                        ht_tiles[ci][:, m * ln : (m + 1) * ln],
                        start=(m == 0),
                        stop=(m == MH - 1),
                    )
                nc.scalar.activation(
                    z_sb[:, off : off + ln],
                    pz[:],
                    mybir.ActivationFunctionType.Identity,
                    bias=b2sum[:],
                )
                nc.vector.tensor_mul(
                    z_sb[:, off : off + ln],
                    z_sb[:, off : off + ln],
                    g_sb[:, off : off + ln],
                )
            nc.sync.dma_start(out=z_d[:], in_=z_sb[:])

    nc.compile()
    _BUILD_CACHE[key] = nc
    return nc


def kernel(x, wg, w1, b1, w2, b2, k):
    from concourse.bass_utils import run_bass_kernel_spmd

    x = np.asarray(x)
    wg = np.asarray(wg)
    w1 = np.asarray(w1)
    b1 = np.asarray(b1)
    w2 = np.asarray(w2)
    b2 = np.asarray(b2)
    k = int(k)

    B, S, D = x.shape
    E = wg.shape[1]
    H = w1.shape[2]
    T = B * S
    assert E == N_CORES, f"expert-parallel layout assumes E == 8, got {E}"

    xf = np.ascontiguousarray(x.reshape(T, D), dtype=np.float32)

    # --- gate + top-k routing (host; needed to build the dispatch shards) ---
    logits = xf @ wg.astype(np.float32)
    logits -= logits.max(axis=1, keepdims=True)
    np.exp(logits, out=logits)
    scores = logits / logits.sum(axis=1, keepdims=True)
    if k >= E:
        topi = np.broadcast_to(np.arange(E, dtype=np.int64), (T, E))
    else:
        topi = np.argpartition(-scores, k, axis=1)[:, :k]
    rows = np.arange(T)[:, None]
    topv = scores[rows, topi]

    # per-expert token lists
    idx_e = []
    val_e = []
    for e in range(E):
        tmask, kpos = np.nonzero(topi == e)
        idx_e.append(tmask)
        val_e.append(topv[tmask, kpos].astype(np.float32))
    max_cnt = max(len(i) for i in idx_e)
    C = max(512, _round_up(max_cnt, 256))

    nc = _build_program(D, H, C, w2.shape[2])

    in_maps = []
    for e in range(E):
        n_e = len(idx_e[e])
        xtg = np.zeros((D, C), dtype=np.float32)
        xtg[:, :n_e] = xf[idx_e[e]].T
        g = np.zeros((1, C), dtype=np.float32)
        g[0, :n_e] = val_e[e]
        b1t = np.ascontiguousarray(
            b1[e].astype(np.float32).reshape(H // P, P).T
        )
        in_maps.append(
            {
                "xtg": xtg,
                "w1": np.ascontiguousarray(w1[e], dtype=np.float32),
                "b1t": b1t,
                "w2": np.ascontiguousarray(w2[e], dtype=np.float32),
                "b2": np.ascontiguousarray(b2[e][None, :], dtype=np.float32),
                "g": g,
            }
        )

    res = run_bass_kernel_spmd(nc, in_maps, core_ids=list(range(N_CORES)))

    # --- combine: scatter-add per-(token, expert) scalars, then log_softmax ---
    s = np.zeros(T, dtype=np.float32)
    for e in range(E):
        n_e = len(idx_e[e])
        if n_e:
            s[idx_e[e]] += res.results[e]["z"][0, :n_e]

    sm = s.reshape(B, S)
    sm = sm - sm.max(axis=1, keepdims=True)
    out = sm - np.log(np.exp(sm).sum(axis=1, keepdims=True))
    return out.astype(np.float32)


# revision 27
# speedup vs baseline: 1.4107x; 1.0136x over previous
"""Expert-parallel MoE routing kernel for Trainium2 (8 NeuronCores).

Problem: top-k(=2) softmax-gated MoE FFN (relu), followed by
log_softmax(sum(moe_out, axis=-1)) over the sequence dim.

Key algebraic observation: the graded output is
    log_softmax_S( sum_d moe_out[t, d] )
and
    sum_d moe_out[t, :] = sum_e combine[t,e] * (relu(x_t @ W1_e + b1_e) @ rowsum(W2_e) + sum(b2_e))
so the second expert matmul collapses to a matvec against rowsum(W2_e).
All of W2 must still be read from HBM (memory-regime roofline unchanged);
its row-sum is computed on-device by the Vector engine while W1 streams
into the Tensor engine.

Sharding (per the expert-parallel hint): core e owns expert e's weights.
The host computes the (tiny) gate/top-k routing to build the dispatch
(it must, to construct the per-core input shards), gathers each expert's
tokens, and the device does the entire FFN including gate-value scaling.
Host then scatter-adds the per-(token,expert) scalars and applies the
final log_softmax on the [B, S] result.

Matmuls run as float32r (fp32 storage, reduced-precision PE mode, 4x the
throughput of strict fp32; measured rel-err ~2e-4 end to end).
"""

import os

import numpy as np

N_CORES = 8
P = 128


def _round_up(v, m):
    return ((v + m - 1) // m) * m


def _chunks(C):
    # even halves keep every matmul's streaming time >= the ~100ns weight
    # load, so LDWEIGHTS stays hidden behind the previous matmul (and f32r
    # needs each chunk >= 256 for its fast rate)
    if C <= 512:
        return [(0, C)]
    assert C % 256 == 0
    h = C // 2
    return [(0, h), (h, h)]


_BUILD_CACHE = {}

# matmul operand dtype: "bf16" (1 cy/row, in-flight cast on DMA, ~3e-3 rel err)
# or "f32r" (fp32 storage, ~2.8 cy/row, ~1e-4 rel err)
MM_MODE = os.environ.get("MOE_MM_MODE", "bf16")


def _build_program(D, H, C, n_b2):
    """Trace + compile the single-core program (SPMD across 8 cores).

    Per-core inputs:
      xtg [D, C]  f32r  gathered tokens for this expert, transposed
      w1  [D, H]  f32r  expert's first-layer weight (natural = lhsT layout)
      b1t [P, H/P] f32  expert's first-layer bias, column m = b1[m*128:(m+1)*128]
      w2  [H, n_b2] f32 expert's second-layer weight
      b2  [1, n_b2] f32 expert's second-layer bias
      g   [1, C]  f32   gate values per slot (0 for padding slots)
    Output:
      z [1, C] f32 = g * (relu(x @ w1 + b1) @ rowsum(w2) + sum(b2))
    """
    key = (D, H, C, n_b2, MM_MODE)
    if key in _BUILD_CACHE:
        return _BUILD_CACHE[key]

    import concourse.tile as tile
    from concourse import bacc, mybir

    f32 = mybir.dt.float32
    mmdt = mybir.dt.bfloat16 if MM_MODE == "bf16" else mybir.dt.float32r
    KD = D // P  # k-tiles over D
    MH = H // P  # m-tiles over H
    chunks = _chunks(C)

    nc = bacc.Bacc("TRN2", target_bir_lowering=False, debug=False)
    in_dt = f32 if MM_MODE == "bf16" else mybir.dt.float32r
    xtg_d = nc.dram_tensor("xtg", [D, C], in_dt, kind="ExternalInput").ap()
    w1_d = nc.dram_tensor("w1", [D, H], in_dt, kind="ExternalInput").ap()
    b1t_d = nc.dram_tensor("b1t", [P, MH], f32, kind="ExternalInput").ap()
    w2_d = nc.dram_tensor("w2", [H, n_b2], f32, kind="ExternalInput").ap()
    b2_d = nc.dram_tensor("b2", [1, n_b2], f32, kind="ExternalInput").ap()
    g_d = nc.dram_tensor("g", [1, C], f32, kind="ExternalInput").ap()
    z_d = nc.dram_tensor("z", [1, C], f32, kind="ExternalOutput").ap()

    cast_needed = MM_MODE == "bf16"

    with tile.TileContext(nc) as tc:
        with (
            tc.tile_pool(name="persist", bufs=1) as persist,
            tc.tile_pool(name="stage", bufs=14) as stage,
            tc.tile_pool(name="w2p", bufs=6) as w2p,
            tc.tile_pool(name="psum_h", bufs=6, space="PSUM") as psum_h,
            tc.tile_pool(name="psum_z", bufs=2, space="PSUM") as psum_z,
        ):
            def load_cast(dst_ap, src_ap, stage_shape, eng=None, tag="stage"):
                # fast HWDGE fp32 load + engine cast copy; plain HWDGE load
                # when no cast is needed
                if cast_needed:
                    st = stage.tile(stage_shape, f32, tag=tag, name=tag)
                    nc.sync.dma_start(out=st[:], in_=src_ap)
                    if eng is nc.scalar:
                        nc.scalar.copy(out=dst_ap, in_=st[:])
                    else:
                        (eng or nc.vector).tensor_copy(out=dst_ap, in_=st[:])
                else:
                    nc.sync.dma_start(out=dst_ap, in_=src_ap)

            # --- small loads first: a few KB, and the relu bias (b1t) gates
            # PSUM recycling so it must not queue behind the weight stream ---
            g_sb = persist.tile([1, C], f32)
            nc.sync.dma_start(out=g_sb[:], in_=g_d[:])
            b1t_sb = persist.tile([P, MH], f32)
            nc.sync.dma_start(out=b1t_sb[:], in_=b1t_d[:])
            b2_sb = persist.tile([1, n_b2], f32)
            nc.sync.dma_start(out=b2_sb[:], in_=b2_d[:])

            # --- activations (xtg, [128, C] per D-tile) and w1 (resident
            # k-major tiles, loaded in 512-column pieces = one m-group).
            # Issue order puts group 0's dependencies first: xtg[0] and the
            # first w1 piece, then the rest of xtg on the (idle) Scalar
            # engine while the Vector engine casts the w1 stream. The mm1
            # kd-loop consumes xtg[kd] progressively, so later xtg tiles
            # may land after the first matmuls have started. ---
            GRP = 4
            PIECE = GRP * P

            def xtg_load(kd, eng):
                t = persist.tile([P, C], mmdt, tag=f"xtg{kd}", name=f"xtg{kd}")
                load_cast(
                    t[:], xtg_d[kd * P : (kd + 1) * P, :], [P, C], eng=eng,
                    tag="xstage",
                )
                return t

            def w1_piece(kd, p0):
                sl = slice(p0, p0 + PIECE)
                load_cast(
                    w1_tiles[kd][:, sl], w1_d[kd * P : (kd + 1) * P, sl], [P, PIECE]
                )

            w1_tiles = [
                persist.tile([P, H], mmdt, tag=f"w1k{kd}", name=f"w1k{kd}")
                for kd in range(KD)
            ]
            xtg_tiles = [None] * KD
            xtg_tiles[0] = xtg_load(0, nc.vector)
            xtg_tiles[1] = xtg_load(1, nc.scalar)
            for kd in range(KD):
                w1_piece(kd, 0)
            for kd in range(2, KD):
                xtg_tiles[kd] = xtg_load(kd, nc.scalar)
            for p0 in range(PIECE, H, PIECE):
                for kd in range(KD):
                    w1_piece(kd, p0)

            b2sum = persist.tile([1, 1], f32)
            nc.vector.reduce_sum(out=b2sum[:], in_=b2_sb[:], axis=mybir.AxisListType.X)

            # w2 row-sums, column m = rowsum over free dim of w2 m-tile
            w2sum = persist.tile([P, MH], mmdt)
            # hT per chunk: [128, MH * chunk_len], slice m holds h^T m-tile
            ht_tiles = [
                persist.tile([P, MH * ln], mmdt, tag=f"ht{ci}", name=f"ht{ci}")
                for ci, (off, ln) in enumerate(chunks)
            ]

            # --- mm1 + relu: h^T[m-tile] = relu(w1^T x^T + b1) ---
            # m-groups of GRP (aligned to the w1 DMA pieces) with the kd
            # accumulation loop outer-per-group: GRP open PSUM accumulators,
            # so the Tensor engine starts on group 0 right after xtg and the
            # first w1 piece land, and both C-chunks run while the group's
            # weights are resident
            for m0 in range(0, MH, GRP):
                grp = range(m0, min(m0 + GRP, MH))
                for ci, (off, ln) in enumerate(chunks):
                    pss = {}
                    for m in grp:
                        pss[m] = psum_h.tile([P, ln], f32, tag="psh", name="psh")
                    for kd in range(KD):
                        for m in grp:
                            nc.tensor.matmul(
                                pss[m][:],
                                w1_tiles[kd][:, m * P : (m + 1) * P],
                                xtg_tiles[kd][:, off : off + ln],
                                start=(kd == 0),
                                stop=(kd == KD - 1),
                            )
                    for m in grp:
                        nc.scalar.activation(
                            ht_tiles[ci][:, m * ln : (m + 1) * ln],
                            pss[m][:],
                            mybir.ActivationFunctionType.Relu,
                            bias=b1t_sb[:, m : m + 1],
                        )

            # --- w2 stream (DVE row-sums, overlaps the PE loop) ---
            # issued on the sync queue AFTER w1/xtg so it does not steal
            # early HBM bandwidth; w2sum is only needed by the matvec at
            # the very end
            for m in range(MH):
                w2t = w2p.tile([P, n_b2], f32)
                nc.sync.dma_start(out=w2t[:], in_=w2_d[m * P : (m + 1) * P, :])
                with nc.allow_low_precision(
                    reason="w2 row-sum feeds reduced-precision matmul anyway"
                ):
                    nc.vector.reduce_sum(
                        out=w2sum[:, m : m + 1], in_=w2t[:], axis=mybir.AxisListType.X
                    )

            # --- matvec against w2 row-sums + bias + gate scale ---
            z_sb = persist.tile([1, C], f32)
            for ci, (off, ln) in enumerate(chunks):
                pz = psum_z.tile([1, ln], f32)
                for m in range(MH):
                    nc.tensor.matmul(
                        pz[:],
                        w2sum[:, m : m + 1],
                        ht_tiles[ci][:, m * ln : (m + 1) * ln],
                        start=(m == 0),
                        stop=(m == MH - 1),
                    )
                nc.scalar.activation(
                    z_sb[:, off : off + ln],
                    pz[:],
                    mybir.ActivationFunctionType.Identity,
                    bias=b2sum[:],
                )
                nc.vector.tensor_mul(
                    z_sb[:, off : off + ln],
                    z_sb[:, off : off + ln],
                    g_sb[:, off : off + ln],
                )
            nc.sync.dma_start(out=z_d[:], in_=z_sb[:])

    nc.compile()
    _BUILD_CACHE[key] = nc
    return nc


def kernel(x, wg, w1, b1, w2, b2, k):
    from concourse.bass_utils import run_bass_kernel_spmd

    x = np.asarray(x)
    wg = np.asarray(wg)
    w1 = np.asarray(w1)
    b1 = np.asarray(b1)
    w2 = np.asarray(w2)
    b2 = np.asarray(b2)
    k = int(k)

    B, S, D = x.shape
    E = wg.shape[1]
    H = w1.shape[2]
    T = B * S
    assert E == N_CORES, f"expert-parallel layout assumes E == 8, got {E}"

    xf = np.ascontiguousarray(x.reshape(T, D), dtype=np.float32)

    # --- gate + top-k routing (host; needed to build the dispatch shards) ---
    logits = xf @ wg.astype(np.float32)
    logits -= logits.max(axis=1, keepdims=True)
    np.exp(logits, out=logits)
    scores = logits / logits.sum(axis=1, keepdims=True)
    if k >= E:
        topi = np.broadcast_to(np.arange(E, dtype=np.int64), (T, E))
    else:
        topi = np.argpartition(-scores, k, axis=1)[:, :k]
    rows = np.arange(T)[:, None]
    topv = scores[rows, topi]

    # per-expert token lists
    idx_e = []
    val_e = []
    for e in range(E):
        tmask, kpos = np.nonzero(topi == e)
        idx_e.append(tmask)
        val_e.append(topv[tmask, kpos].astype(np.float32))
    max_cnt = max(len(i) for i in idx_e)
    C = max(512, _round_up(max_cnt, 256))

    nc = _build_program(D, H, C, w2.shape[2])

    in_maps = []
    for e in range(E):
        n_e = len(idx_e[e])
        xtg = np.zeros((D, C), dtype=np.float32)
        xtg[:, :n_e] = xf[idx_e[e]].T
        g = np.zeros((1, C), dtype=np.float32)
        g[0, :n_e] = val_e[e]
        b1t = np.ascontiguousarray(
            b1[e].astype(np.float32).reshape(H // P, P).T
        )
        in_maps.append(
            {
                "xtg": xtg,
                "w1": np.ascontiguousarray(w1[e], dtype=np.float32),
                "b1t": b1t,
                "w2": np.ascontiguousarray(w2[e], dtype=np.float32),
                "b2": np.ascontiguousarray(b2[e][None, :], dtype=np.float32),
                "g": g,
            }
        )

    res = run_bass_kernel_spmd(nc, in_maps, core_ids=list(range(N_CORES)))

    # --- combine: scatter-add per-(token, expert) scalars, then log_softmax ---
    s = np.zeros(T, dtype=np.float32)
    for e in range(E):
        n_e = len(idx_e[e])
        if n_e:
            s[idx_e[e]] += res.results[e]["z"][0, :n_e]

    sm = s.reshape(B, S)
    sm = sm - sm.max(axis=1, keepdims=True)
    out = sm - np.log(np.exp(sm).sum(axis=1, keepdims=True))
    return out.astype(np.float32)


# revision 28
# speedup vs baseline: 1.5039x; 1.0660x over previous
"""Expert-parallel MoE routing kernel for Trainium2 (8 NeuronCores).

Problem: top-k(=2) softmax-gated MoE FFN (relu), followed by
log_softmax(sum(moe_out, axis=-1)) over the sequence dim.

Key algebraic observation: the graded output is
    log_softmax_S( sum_d moe_out[t, d] )
and
    sum_d moe_out[t, :] = sum_e combine[t,e] * (relu(x_t @ W1_e + b1_e) @ rowsum(W2_e) + sum(b2_e))
so the second expert matmul collapses to a matvec against rowsum(W2_e).
All of W2 must still be read from HBM (memory-regime roofline unchanged);
its row-sum is computed on-device by the Vector engine while W1 streams
into the Tensor engine.

Sharding (per the expert-parallel hint): core e owns expert e's weights.
The host computes the (tiny) gate/top-k routing to build the dispatch
(it must, to construct the per-core input shards), gathers each expert's
tokens, and the device does the entire FFN including gate-value scaling.
Host then scatter-adds the per-(token,expert) scalars and applies the
final log_softmax on the [B, S] result.

Matmuls run as float32r (fp32 storage, reduced-precision PE mode, 4x the
throughput of strict fp32; measured rel-err ~2e-4 end to end).
"""

import os

import numpy as np

N_CORES = 8
P = 128


def _round_up(v, m):
    return ((v + m - 1) // m) * m


def _chunks(C):
    # even halves keep every matmul's streaming time >= the ~100ns weight
    # load, so LDWEIGHTS stays hidden behind the previous matmul (and f32r
    # needs each chunk >= 256 for its fast rate)
    if C <= 512:
        return [(0, C)]
    assert C % 256 == 0
    h = C // 2
    return [(0, h), (h, h)]


_BUILD_CACHE = {}

# matmul operand dtype:
#   "f32r": fp32 storage, reduced-precision PE mode -> ~1.2e-4 end-to-end
#           rel err, ~93 us  (default: near-identical speed, 22x accuracy)
#   "bf16": staged DVE/ACT casts, fp32 HBM traffic  -> ~2.7e-3 rel err, ~86 us
# Both are bounded by the same ~190-200 ns weight-load+matmul pair cost.
MM_MODE = os.environ.get("MOE_MM_MODE", "f32r")


def _build_program(D, H, C, n_b2):
    """Trace + compile the single-core program (SPMD across 8 cores).

    Per-core inputs:
      xtg [D, C]  f32r  gathered tokens for this expert, transposed
      w1  [D, H]  f32r  expert's first-layer weight (natural = lhsT layout)
      b1t [P, H/P] f32  expert's first-layer bias, column m = b1[m*128:(m+1)*128]
      w2  [H, n_b2] f32 expert's second-layer weight
      b2  [1, n_b2] f32 expert's second-layer bias
      g   [1, C]  f32   gate values per slot (0 for padding slots)
    Output:
      z [1, C] f32 = g * (relu(x @ w1 + b1) @ rowsum(w2) + sum(b2))
    """
    key = (D, H, C, n_b2, MM_MODE)
    if key in _BUILD_CACHE:
        return _BUILD_CACHE[key]

    import concourse.tile as tile
    from concourse import bacc, mybir

    f32 = mybir.dt.float32
    mmdt = mybir.dt.bfloat16 if MM_MODE == "bf16" else mybir.dt.float32r
    KD = D // P  # k-tiles over D
    MH = H // P  # m-tiles over H
    chunks = _chunks(C)

    nc = bacc.Bacc("TRN2", target_bir_lowering=False, debug=False)
    in_dt = f32 if MM_MODE == "bf16" else mybir.dt.float32r
    xtg_d = nc.dram_tensor("xtg", [D, C], in_dt, kind="ExternalInput").ap()
    w1_d = nc.dram_tensor("w1", [D, H], in_dt, kind="ExternalInput").ap()
    b1t_d = nc.dram_tensor("b1t", [P, MH], f32, kind="ExternalInput").ap()
    w2_d = nc.dram_tensor("w2", [H, n_b2], f32, kind="ExternalInput").ap()
    b2_d = nc.dram_tensor("b2", [1, n_b2], f32, kind="ExternalInput").ap()
    g_d = nc.dram_tensor("g", [1, C], f32, kind="ExternalInput").ap()
    z_d = nc.dram_tensor("z", [1, C], f32, kind="ExternalOutput").ap()

    cast_needed = MM_MODE == "bf16"

    with tile.TileContext(nc) as tc:
        with (
            tc.tile_pool(name="persist", bufs=1) as persist,
            tc.tile_pool(name="stage", bufs=14) as stage,
            tc.tile_pool(name="w2p", bufs=6) as w2p,
            tc.tile_pool(name="psum_h", bufs=6, space="PSUM") as psum_h,
            tc.tile_pool(name="psum_z", bufs=2, space="PSUM") as psum_z,
        ):
            def load_cast(dst_ap, src_ap, stage_shape, eng=None, tag="stage"):
                # fast HWDGE fp32 load + engine cast copy; plain HWDGE load
                # when no cast is needed
                if cast_needed:
                    st = stage.tile(stage_shape, f32, tag=tag, name=tag)
                    nc.sync.dma_start(out=st[:], in_=src_ap)
                    if eng is nc.scalar:
                        nc.scalar.copy(out=dst_ap, in_=st[:])
                    else:
                        (eng or nc.vector).tensor_copy(out=dst_ap, in_=st[:])
                else:
                    nc.sync.dma_start(out=dst_ap, in_=src_ap)

            # --- small loads first: a few KB, and the relu bias (b1t) gates
            # PSUM recycling so it must not queue behind the weight stream ---
            g_sb = persist.tile([1, C], f32)
            nc.sync.dma_start(out=g_sb[:], in_=g_d[:])
            b1t_sb = persist.tile([P, MH], f32)
            nc.sync.dma_start(out=b1t_sb[:], in_=b1t_d[:])
            b2_sb = persist.tile([1, n_b2], f32)
            nc.sync.dma_start(out=b2_sb[:], in_=b2_d[:])

            # --- activations (xtg, [128, C] per D-tile) and w1 (resident
            # k-major tiles, loaded in 512-column pieces = one m-group).
            # Issue order puts group 0's dependencies first: xtg[0] and the
            # first w1 piece, then the rest of xtg on the (idle) Scalar
            # engine while the Vector engine casts the w1 stream. The mm1
            # kd-loop consumes xtg[kd] progressively, so later xtg tiles
            # may land after the first matmuls have started. ---
            GRP = 4
            PIECE = GRP * P

            def xtg_load(kd, eng):
                t = persist.tile([P, C], mmdt, tag=f"xtg{kd}", name=f"xtg{kd}")
                load_cast(
                    t[:], xtg_d[kd * P : (kd + 1) * P, :], [P, C], eng=eng,
                    tag="xstage",
                )
                return t

            def w1_piece(kd, p0):
                sl = slice(p0, p0 + PIECE)
                load_cast(
                    w1_tiles[kd][:, sl], w1_d[kd * P : (kd + 1) * P, sl], [P, PIECE]
                )

            w1_tiles = [
                persist.tile([P, H], mmdt, tag=f"w1k{kd}", name=f"w1k{kd}")
                for kd in range(KD)
            ]
            xtg_tiles = [None] * KD
            xtg_tiles[0] = xtg_load(0, nc.vector)
            xtg_tiles[1] = xtg_load(1, nc.scalar)
            for kd in range(KD):
                w1_piece(kd, 0)
            for kd in range(2, KD):
                xtg_tiles[kd] = xtg_load(kd, nc.scalar)
            for p0 in range(PIECE, H, PIECE):
                for kd in range(KD):
                    w1_piece(kd, p0)

            b2sum = persist.tile([1, 1], f32)
            nc.vector.reduce_sum(out=b2sum[:], in_=b2_sb[:], axis=mybir.AxisListType.X)

            # w2 row-sums, column m = rowsum over free dim of w2 m-tile
            w2sum = persist.tile([P, MH], mmdt)
            # hT per chunk: [128, MH * chunk_len], slice m holds h^T m-tile
            ht_tiles = [
                persist.tile([P, MH * ln], mmdt, tag=f"ht{ci}", name=f"ht{ci}")
                for ci, (off, ln) in enumerate(chunks)
            ]

            # --- mm1 + relu: h^T[m-tile] = relu(w1^T x^T + b1) ---
            # m-groups of GRP (aligned to the w1 DMA pieces) with the kd
            # accumulation loop outer-per-group: GRP open PSUM accumulators,
            # so the Tensor engine starts on group 0 right after xtg and the
            # first w1 piece land, and both C-chunks run while the group's
            # weights are resident
            for m0 in range(0, MH, GRP):
                grp = range(m0, min(m0 + GRP, MH))
                for ci, (off, ln) in enumerate(chunks):
                    pss = {}
                    for m in grp:
                        pss[m] = psum_h.tile([P, ln], f32, tag="psh", name="psh")
                    for kd in range(KD):
                        for m in grp:
                            nc.tensor.matmul(
                                pss[m][:],
                                w1_tiles[kd][:, m * P : (m + 1) * P],
                                xtg_tiles[kd][:, off : off + ln],
                                start=(kd == 0),
                                stop=(kd == KD - 1),
                            )
                    for m in grp:
                        nc.scalar.activation(
                            ht_tiles[ci][:, m * ln : (m + 1) * ln],
                            pss[m][:],
                            mybir.ActivationFunctionType.Relu,
                            bias=b1t_sb[:, m : m + 1],
                        )

            # --- w2 stream (DVE row-sums, overlaps the PE loop) ---
            # issued on the sync queue AFTER w1/xtg so it does not steal
            # early HBM bandwidth; w2sum is only needed by the matvec at
            # the very end
            for m in range(MH):
                w2t = w2p.tile([P, n_b2], f32)
                nc.sync.dma_start(out=w2t[:], in_=w2_d[m * P : (m + 1) * P, :])
                with nc.allow_low_precision(
                    reason="w2 row-sum feeds reduced-precision matmul anyway"
                ):
                    nc.vector.reduce_sum(
                        out=w2sum[:, m : m + 1], in_=w2t[:], axis=mybir.AxisListType.X
                    )

            # --- matvec against w2 row-sums + bias + gate scale ---
            z_sb = persist.tile([1, C], f32)
            for ci, (off, ln) in enumerate(chunks):
                pz = psum_z.tile([1, ln], f32)
                for m in range(MH):
                    nc.tensor.matmul(
                        pz[:],
                        w2sum[:, m : m + 1],
                        ht_tiles[ci][:, m * ln : (m + 1) * ln],
                        start=(m == 0),
                        stop=(m == MH - 1),
                    )
                nc.scalar.activation(
                    z_sb[:, off : off + ln],
                    pz[:],
                    mybir.ActivationFunctionType.Identity,
                    bias=b2sum[:],
                )
                nc.vector.tensor_mul(
                    z_sb[:, off : off + ln],
                    z_sb[:, off : off + ln],
                    g_sb[:, off : off + ln],
                )
            nc.sync.dma_start(out=z_d[:], in_=z_sb[:])

    nc.compile()
    _BUILD_CACHE[key] = nc
    return nc


def kernel(x, wg, w1, b1, w2, b2, k):
    from concourse.bass_utils import run_bass_kernel_spmd

    x = np.asarray(x)
    wg = np.asarray(wg)
    w1 = np.asarray(w1)
    b1 = np.asarray(b1)
    w2 = np.asarray(w2)
    b2 = np.asarray(b2)
    k = int(k)

    B, S, D = x.shape
    E = wg.shape[1]
    H = w1.shape[2]
    T = B * S
    assert E == N_CORES, f"expert-parallel layout assumes E == 8, got {E}"

    xf = np.ascontiguousarray(x.reshape(T, D), dtype=np.float32)

    # --- gate + top-k routing (host; needed to build the dispatch shards) ---
    logits = xf @ wg.astype(np.float32)
    logits -= logits.max(axis=1, keepdims=True)
    np.exp(logits, out=logits)
    scores = logits / logits.sum(axis=1, keepdims=True)
    if k >= E:
        topi = np.broadcast_to(np.arange(E, dtype=np.int64), (T, E))
    else:
        topi = np.argpartition(-scores, k, axis=1)[:, :k]
    rows = np.arange(T)[:, None]
    topv = scores[rows, topi]

    # per-expert token lists
    idx_e = []
    val_e = []
    for e in range(E):
        tmask, kpos = np.nonzero(topi == e)
        idx_e.append(tmask)
        val_e.append(topv[tmask, kpos].astype(np.float32))
    max_cnt = max(len(i) for i in idx_e)
    C = max(512, _round_up(max_cnt, 256))

    nc = _build_program(D, H, C, w2.shape[2])

    in_maps = []
    for e in range(E):
        n_e = len(idx_e[e])
        xtg = np.zeros((D, C), dtype=np.float32)
        xtg[:, :n_e] = xf[idx_e[e]].T
        g = np.zeros((1, C), dtype=np.float32)
        g[0, :n_e] = val_e[e]
        b1t = np.ascontiguousarray(
            b1[e].astype(np.float32).reshape(H // P, P).T
        )
        in_maps.append(
            {
                "xtg": xtg,
                "w1": np.ascontiguousarray(w1[e], dtype=np.float32),
                "b1t": b1t,
                "w2": np.ascontiguousarray(w2[e], dtype=np.float32),
                "b2": np.ascontiguousarray(b2[e][None, :], dtype=np.float32),
                "g": g,
            }
        )

    res = run_bass_kernel_spmd(nc, in_maps, core_ids=list(range(N_CORES)))

    # --- combine: scatter-add per-(token, expert) scalars, then log_softmax ---
    s = np.zeros(T, dtype=np.float32)
    for e in range(E):
        n_e = len(idx_e[e])
        if n_e:
            s[idx_e[e]] += res.results[e]["z"][0, :n_e]

    sm = s.reshape(B, S)
    sm = sm - sm.max(axis=1, keepdims=True)
    out = sm - np.log(np.exp(sm).sum(axis=1, keepdims=True))
    return out.astype(np.float32)


# revision 35
# speedup vs baseline: 1.5375x; 1.0223x over previous
"""Expert-parallel MoE routing kernel for Trainium2 (8 NeuronCores).

Problem: top-k(=2) softmax-gated MoE FFN (relu), followed by
log_softmax(sum(moe_out, axis=-1)) over the sequence dim.

Key algebraic observation: the graded output is
    log_softmax_S( sum_d moe_out[t, d] )
and
    sum_d moe_out[t, :] = sum_e combine[t,e] * (relu(x_t @ W1_e + b1_e) @ rowsum(W2_e) + sum(b2_e))
so the second expert matmul collapses to a matvec against rowsum(W2_e).
All of W2 must still be read from HBM (memory-regime roofline unchanged);
its row-sum is computed on-device by the Vector engine while W1 streams
into the Tensor engine.

Sharding (per the expert-parallel hint): core e owns expert e's weights.
The host computes the (tiny) gate/top-k routing to build the dispatch
(it must, to construct the per-core input shards), gathers each expert's
tokens, and the device does the entire FFN including gate-value scaling.
Host then scatter-adds the per-(token,expert) scalars and applies the
final log_softmax on the [B, S] result.

Matmuls default to float32r (fp32 storage, reduced-precision PE mode;
measured ~1.2e-4 end-to-end rel err). MOE_MM_MODE=bf16 selects bf16
operands (in-SBUF casts, fp32 HBM traffic, ~2.7e-3 rel err) at nearly
the same speed — both are bound by HBM reads plus the per-matmul
weight-load pipeline.
"""

import os

import numpy as np

N_CORES = 8
P = 128


def _round_up(v, m):
    return ((v + m - 1) // m) * m


def _chunks(C):
    # even-ish pieces of 256..512 (fp32 moving-operand max is 512, one PSUM
    # bank per matmul); keeping every chunk >= 256 keeps each matmul's
    # streaming time above the ~100-190ns weight load and f32r at its fast
    # rate
    if C <= 512:
        return [(0, C)]
    n = -(-C // 512)
    base, extra = divmod(C, n)
    out = []
    off = 0
    for i in range(n):
        ln = base + (1 if i < extra else 0)
        out.append((off, ln))
        off += ln
    return out


_BUILD_CACHE = {}

# matmul operand dtype:
#   "f32r": fp32 storage, reduced-precision PE mode -> ~1.2e-4 end-to-end
#           rel err, ~93 us  (default: near-identical speed, 22x accuracy)
#   "bf16": staged DVE/ACT casts, fp32 HBM traffic  -> ~2.7e-3 rel err, ~86 us
# Both are bounded by the same ~190-200 ns weight-load+matmul pair cost.
MM_MODE = os.environ.get("MOE_MM_MODE", "f32r")


def _build_program(D, H, C, n_b2):
    """Trace + compile the single-core program (SPMD across 8 cores).

    Per-core inputs:
      xtg [D, C]  f32r  gathered tokens for this expert, transposed
      w1  [D, H]  f32r  expert's first-layer weight (natural = lhsT layout)
      b1t [P, H/P] f32  expert's first-layer bias, column m = b1[m*128:(m+1)*128]
      w2  [H, n_b2] f32 expert's second-layer weight
      b2  [1, n_b2] f32 expert's second-layer bias
      g   [1, C]  f32   gate values per slot (0 for padding slots)
    Output:
      z [1, C] f32 = g * (relu(x @ w1 + b1) @ rowsum(w2) + sum(b2))
    """
    key = (D, H, C, n_b2, MM_MODE)
    if key in _BUILD_CACHE:
        return _BUILD_CACHE[key]

    import concourse.tile as tile
    from concourse import bacc, mybir

    f32 = mybir.dt.float32
    mmdt = mybir.dt.bfloat16 if MM_MODE == "bf16" else mybir.dt.float32r
    KD = D // P  # k-tiles over D
    MH = H // P  # m-tiles over H
    chunks = _chunks(C)

    nc = bacc.Bacc("TRN2", target_bir_lowering=False, debug=False)
    in_dt = f32 if MM_MODE == "bf16" else mybir.dt.float32r
    xtg_d = nc.dram_tensor("xtg", [D, C], in_dt, kind="ExternalInput").ap()
    w1_d = nc.dram_tensor("w1", [D, H], in_dt, kind="ExternalInput").ap()
    b1t_d = nc.dram_tensor("b1t", [P, MH], f32, kind="ExternalInput").ap()
    w2_d = nc.dram_tensor("w2", [H, n_b2], f32, kind="ExternalInput").ap()
    b2_d = nc.dram_tensor("b2", [1, n_b2], f32, kind="ExternalInput").ap()
    g_d = nc.dram_tensor("g", [1, C], f32, kind="ExternalInput").ap()
    z_d = nc.dram_tensor("z", [1, C], f32, kind="ExternalOutput").ap()

    cast_needed = MM_MODE == "bf16"

    with tile.TileContext(nc) as tc:
        with (
            tc.tile_pool(name="persist", bufs=1) as persist,
            tc.tile_pool(name="stage", bufs=14) as stage,
            tc.tile_pool(name="w2p", bufs=6) as w2p,
            tc.tile_pool(name="psum_h", bufs=6, space="PSUM") as psum_h,
            tc.tile_pool(name="psum_z", bufs=2, space="PSUM") as psum_z,
        ):
            def load_cast(dst_ap, src_ap, stage_shape, eng=None, tag="stage"):
                # fast HWDGE fp32 load + engine cast copy; plain HWDGE load
                # when no cast is needed
                if cast_needed:
                    st = stage.tile(stage_shape, f32, tag=tag, name=tag)
                    nc.sync.dma_start(out=st[:], in_=src_ap)
                    if eng is nc.scalar:
                        nc.scalar.copy(out=dst_ap, in_=st[:])
                    else:
                        (eng or nc.vector).tensor_copy(out=dst_ap, in_=st[:])
                else:
                    nc.sync.dma_start(out=dst_ap, in_=src_ap)

            # --- small loads first: a few KB, and the relu bias (b1t) gates
            # PSUM recycling so it must not queue behind the weight stream ---
            g_sb = persist.tile([1, C], f32)
            nc.sync.dma_start(out=g_sb[:], in_=g_d[:])
            b1t_sb = persist.tile([P, MH], f32)
            nc.sync.dma_start(out=b1t_sb[:], in_=b1t_d[:])
            b2_sb = persist.tile([1, n_b2], f32)
            nc.sync.dma_start(out=b2_sb[:], in_=b2_d[:])

            # --- activations (xtg, [128, C] per D-tile) and w1 (resident
            # k-major tiles, loaded in 512-column pieces = one m-group).
            # Issue order puts group 0's dependencies first: xtg[0] and the
            # first w1 piece, then the rest of xtg on the (idle) Scalar
            # engine while the Vector engine casts the w1 stream. The mm1
            # kd-loop consumes xtg[kd] progressively, so later xtg tiles
            # may land after the first matmuls have started. ---
            GRP = 4
            PIECE = GRP * P

            def xtg_load(kd, eng):
                t = persist.tile([P, C], mmdt, tag=f"xtg{kd}", name=f"xtg{kd}")
                load_cast(
                    t[:], xtg_d[kd * P : (kd + 1) * P, :], [P, C], eng=eng,
                    tag="xstage",
                )
                return t

            def w1_piece(kd, p0):
                ln = min(PIECE, H - p0)
                sl = slice(p0, p0 + ln)
                load_cast(
                    w1_tiles[kd][:, sl], w1_d[kd * P : (kd + 1) * P, sl], [P, ln]
                )

            w1_tiles = [
                persist.tile([P, H], mmdt, tag=f"w1k{kd}", name=f"w1k{kd}")
                for kd in range(KD)
            ]
            xtg_tiles = [None] * KD
            xtg_tiles[0] = xtg_load(0, nc.vector)
            xtg_tiles[1] = xtg_load(1, nc.scalar)
            for kd in range(KD):
                w1_piece(kd, 0)
            for kd in range(2, KD):
                xtg_tiles[kd] = xtg_load(kd, nc.scalar)
            for p0 in range(PIECE, H, PIECE):
                for kd in range(KD):
                    w1_piece(kd, p0)

            b2sum = persist.tile([1, 1], f32)
            nc.vector.reduce_sum(out=b2sum[:], in_=b2_sb[:], axis=mybir.AxisListType.X)

            # w2 row-sums, column m = rowsum over free dim of w2 m-tile
            w2sum = persist.tile([P, MH], mmdt)
            # hT per chunk: [128, MH * chunk_len], slice m holds h^T m-tile
            ht_tiles = [
                persist.tile([P, MH * ln], mmdt, tag=f"ht{ci}", name=f"ht{ci}")
                for ci, (off, ln) in enumerate(chunks)
            ]

            # --- mm1 + relu: h^T[m-tile] = relu(w1^T x^T + b1) ---
            # m-groups of GRP (aligned to the w1 DMA pieces) with the kd
            # accumulation loop outer-per-group: GRP open PSUM accumulators,
            # so the Tensor engine starts on group 0 right after xtg and the
            # first w1 piece land, and both C-chunks run while the group's
            # weights are resident
            for m0 in range(0, MH, GRP):
                grp = range(m0, min(m0 + GRP, MH))
                for ci, (off, ln) in enumerate(chunks):
                    pss = {}
                    for m in grp:
                        pss[m] = psum_h.tile([P, ln], f32, tag="psh", name="psh")
                    for kd in range(KD):
                        for m in grp:
                            nc.tensor.matmul(
                                pss[m][:],
                                w1_tiles[kd][:, m * P : (m + 1) * P],
                                xtg_tiles[kd][:, off : off + ln],
                                start=(kd == 0),
                                stop=(kd == KD - 1),
                            )
                    for m in grp:
                        nc.scalar.activation(
                            ht_tiles[ci][:, m * ln : (m + 1) * ln],
                            pss[m][:],
                            mybir.ActivationFunctionType.Relu,
                            bias=b1t_sb[:, m : m + 1],
                        )

            # --- w2 stream (DVE row-sums, overlaps the PE loop) ---
            # issued on the sync queue AFTER w1/xtg so it does not steal
            # early HBM bandwidth; w2sum is only needed by the matvec at
            # the very end
            for m in range(MH):
                w2t = w2p.tile([P, n_b2], f32)
                nc.sync.dma_start(out=w2t[:], in_=w2_d[m * P : (m + 1) * P, :])
                with nc.allow_low_precision(
                    reason="w2 row-sum feeds reduced-precision matmul anyway"
                ):
                    nc.vector.reduce_sum(
                        out=w2sum[:, m : m + 1], in_=w2t[:], axis=mybir.AxisListType.X
                    )

            # --- matvec against w2 row-sums + bias + gate scale ---
            z_sb = persist.tile([1, C], f32)
            for ci, (off, ln) in enumerate(chunks):
                pz = psum_z.tile([1, ln], f32)
                for m in range(MH):
                    nc.tensor.matmul(
                        pz[:],
                        w2sum[:, m : m + 1],
                        ht_tiles[ci][:, m * ln : (m + 1) * ln],
                        start=(m == 0),
                        stop=(m == MH - 1),
                    )
                nc.scalar.activation(
                    z_sb[:, off : off + ln],
                    pz[:],
                    mybir.ActivationFunctionType.Identity,
                    bias=b2sum[:],
                )
                nc.vector.tensor_mul(
                    z_sb[:, off : off + ln],
                    z_sb[:, off : off + ln],
                    g_sb[:, off : off + ln],
                )
            nc.sync.dma_start(out=z_d[:], in_=z_sb[:])

    nc.compile()
    _BUILD_CACHE[key] = nc
    return nc


def kernel(x, wg, w1, b1, w2, b2, k):
    from concourse.bass_utils import run_bass_kernel_spmd

    x = np.asarray(x)
    wg = np.asarray(wg)
    w1 = np.asarray(w1)
    b1 = np.asarray(b1)
    w2 = np.asarray(w2)
    b2 = np.asarray(b2)
    k = int(k)

    B, S, D = x.shape
    E = wg.shape[1]
    H = w1.shape[2]
    T = B * S
    assert E == N_CORES, f"expert-parallel layout assumes E == 8, got {E}"
    assert D % P == 0 and H % P == 0, (D, H)

    xf = np.ascontiguousarray(x.reshape(T, D), dtype=np.float32)

    # --- gate + top-k routing (host; needed to build the dispatch shards) ---
    logits = xf @ wg.astype(np.float32)
    logits -= logits.max(axis=1, keepdims=True)
    np.exp(logits, out=logits)
    scores = logits / logits.sum(axis=1, keepdims=True)
    if k >= E:
        topi = np.broadcast_to(np.arange(E, dtype=np.int64), (T, E))
    else:
        topi = np.argpartition(-scores, k, axis=1)[:, :k]
    rows = np.arange(T)[:, None]
    topv = scores[rows, topi]

    # per-expert token lists
    idx_e = []
    val_e = []
    for e in range(E):
        tmask, kpos = np.nonzero(topi == e)
        idx_e.append(tmask)
        val_e.append(topv[tmask, kpos].astype(np.float32))
    max_cnt = max(len(i) for i in idx_e)
    C = max(512, _round_up(max_cnt, 128))

    nc = _build_program(D, H, C, w2.shape[2])

    in_maps = []
    for e in range(E):
        n_e = len(idx_e[e])
        xtg = np.zeros((D, C), dtype=np.float32)
        xtg[:, :n_e] = xf[idx_e[e]].T
        g = np.zeros((1, C), dtype=np.float32)
        g[0, :n_e] = val_e[e]
        b1t = np.ascontiguousarray(
            b1[e].astype(np.float32).reshape(H // P, P).T
        )
        in_maps.append(
            {
                "xtg": xtg,
                "w1": np.ascontiguousarray(w1[e], dtype=np.float32),
                "b1t": b1t,
                "w2": np.ascontiguousarray(w2[e], dtype=np.float32),
                "b2": np.ascontiguousarray(b2[e][None, :], dtype=np.float32),
                "g": g,
            }
        )

    res = run_bass_kernel_spmd(nc, in_maps, core_ids=list(range(N_CORES)))

    # --- combine: scatter-add per-(token, expert) scalars, then log_softmax ---
    s = np.zeros(T, dtype=np.float32)
    for e in range(E):
        n_e = len(idx_e[e])
        if n_e:
            s[idx_e[e]] += res.results[e]["z"][0, :n_e]

    sm = s.reshape(B, S)
    sm = sm - sm.max(axis=1, keepdims=True)
    out = sm - np.log(np.exp(sm).sum(axis=1, keepdims=True))
    return out.astype(np.float32)


# revision 36
# speedup vs baseline: 1.5667x; 1.0190x over previous
"""Expert-parallel MoE routing kernel for Trainium2 (8 NeuronCores).

Problem: top-k(=2) softmax-gated MoE FFN (relu), followed by
log_softmax(sum(moe_out, axis=-1)) over the sequence dim.

Key algebraic observation: the graded output is
    log_softmax_S( sum_d moe_out[t, d] )
and
    sum_d moe_out[t, :] = sum_e combine[t,e] * (relu(x_t @ W1_e + b1_e) @ rowsum(W2_e) + sum(b2_e))
so the second expert matmul collapses to a matvec against rowsum(W2_e).
All of W2 must still be read from HBM (memory-regime roofline unchanged);
its row-sum is computed on-device by the Vector engine while W1 streams
into the Tensor engine.

Sharding (per the expert-parallel hint): core e owns expert e's weights.
The host computes the (tiny) gate/top-k routing to build the dispatch
(it must, to construct the per-core input shards), gathers each expert's
tokens, and the device does the entire FFN including gate-value scaling.
Host then scatter-adds the per-(token,expert) scalars and applies the
final log_softmax on the [B, S] result.

Matmuls default to float32r (fp32 storage, reduced-precision PE mode;
measured ~1.2e-4 end-to-end rel err). MOE_MM_MODE=bf16 selects bf16
operands (in-SBUF casts, fp32 HBM traffic, ~2.7e-3 rel err) at nearly
the same speed — both are bound by HBM reads plus the per-matmul
weight-load pipeline.
"""

import os

import numpy as np

N_CORES = 8
P = 128


def _round_up(v, m):
    return ((v + m - 1) // m) * m


def _chunks(C):
    # even-ish pieces of 256..512 (fp32 moving-operand max is 512, one PSUM
    # bank per matmul); keeping every chunk >= 256 keeps each matmul's
    # streaming time above the ~100-190ns weight load and f32r at its fast
    # rate
    if C <= 512:
        return [(0, C)]
    n = -(-C // 512)
    base, extra = divmod(C, n)
    out = []
    off = 0
    for i in range(n):
        ln = base + (1 if i < extra else 0)
        out.append((off, ln))
        off += ln
    return out


_BUILD_CACHE = {}

# matmul operand dtype:
#   "f32r": fp32 storage, reduced-precision PE mode -> ~1.2e-4 end-to-end
#           rel err, ~93 us  (default: near-identical speed, 22x accuracy)
#   "bf16": staged DVE/ACT casts, fp32 HBM traffic  -> ~2.7e-3 rel err, ~86 us
# Both are bounded by the same ~190-200 ns weight-load+matmul pair cost.
MM_MODE = os.environ.get("MOE_MM_MODE", "f32r")


def _build_program(D, H, C, n_b2):
    """Trace + compile the single-core program (SPMD across 8 cores).

    Per-core inputs:
      xtg [D, C]  f32r  gathered tokens for this expert, transposed
      w1  [D, H]  f32r  expert's first-layer weight (natural = lhsT layout)
      b1t [P, H/P] f32  expert's first-layer bias, column m = b1[m*128:(m+1)*128]
      w2  [H, n_b2] f32 expert's second-layer weight
      b2  [1, n_b2] f32 expert's second-layer bias
      g   [1, C]  f32   gate values per slot (0 for padding slots)
    Output:
      z [1, C] f32 = g * (relu(x @ w1 + b1) @ rowsum(w2) + sum(b2))
    """
    key = (D, H, C, n_b2, MM_MODE)
    if key in _BUILD_CACHE:
        return _BUILD_CACHE[key]

    import concourse.tile as tile
    from concourse import bacc, mybir

    f32 = mybir.dt.float32
    mmdt = mybir.dt.bfloat16 if MM_MODE == "bf16" else mybir.dt.float32r
    KD = D // P  # k-tiles over D
    MH = H // P  # m-tiles over H
    chunks = _chunks(C)

    nc = bacc.Bacc("TRN2", target_bir_lowering=False, debug=False)
    in_dt = f32 if MM_MODE == "bf16" else mybir.dt.float32r
    xtg_d = nc.dram_tensor("xtg", [D, C], in_dt, kind="ExternalInput").ap()
    w1_d = nc.dram_tensor("w1", [D, H], in_dt, kind="ExternalInput").ap()
    b1t_d = nc.dram_tensor("b1t", [P, MH], f32, kind="ExternalInput").ap()
    w2_d = nc.dram_tensor("w2", [H, n_b2], f32, kind="ExternalInput").ap()
    b2_d = nc.dram_tensor("b2", [1, n_b2], f32, kind="ExternalInput").ap()
    g_d = nc.dram_tensor("g", [1, C], f32, kind="ExternalInput").ap()
    z_d = nc.dram_tensor("z", [1, C], f32, kind="ExternalOutput").ap()

    cast_needed = MM_MODE == "bf16"

    with tile.TileContext(nc) as tc:
        with (
            tc.tile_pool(name="persist", bufs=1) as persist,
            tc.tile_pool(name="stage", bufs=14) as stage,
            tc.tile_pool(name="w2p", bufs=6) as w2p,
            tc.tile_pool(name="psum_h", bufs=6, space="PSUM") as psum_h,
            tc.tile_pool(name="psum_z", bufs=2, space="PSUM") as psum_z,
        ):
            def load_cast(dst_ap, src_ap, stage_shape, eng=None, tag="stage"):
                # fast HWDGE fp32 load + engine cast copy; plain HWDGE load
                # when no cast is needed
                if cast_needed:
                    st = stage.tile(stage_shape, f32, tag=tag, name=tag)
                    nc.sync.dma_start(out=st[:], in_=src_ap)
                    if eng is nc.scalar:
                        nc.scalar.copy(out=dst_ap, in_=st[:])
                    else:
                        (eng or nc.vector).tensor_copy(out=dst_ap, in_=st[:])
                else:
                    nc.sync.dma_start(out=dst_ap, in_=src_ap)

            # --- small loads first: a few KB, and the relu bias (b1t) gates
            # PSUM recycling so it must not queue behind the weight stream ---
            g_sb = persist.tile([1, C], f32)
            nc.sync.dma_start(out=g_sb[:], in_=g_d[:])
            b1t_sb = persist.tile([P, MH], f32)
            nc.sync.dma_start(out=b1t_sb[:], in_=b1t_d[:])
            b2_sb = persist.tile([1, n_b2], f32)
            nc.sync.dma_start(out=b2_sb[:], in_=b2_d[:])

            # --- activations (xtg, [128, C] per D-tile) and w1 (resident
            # k-major tiles, loaded in 512-column pieces = one m-group).
            # Issue order puts group 0's dependencies first: xtg[0] and the
            # first w1 piece, then the rest of xtg on the (idle) Scalar
            # engine while the Vector engine casts the w1 stream. The mm1
            # kd-loop consumes xtg[kd] progressively, so later xtg tiles
            # may land after the first matmuls have started. ---
            GRP = 4
            PIECE = GRP * P

            def xtg_load(kd, eng):
                t = persist.tile([P, C], mmdt, tag=f"xtg{kd}", name=f"xtg{kd}")
                load_cast(
                    t[:], xtg_d[kd * P : (kd + 1) * P, :], [P, C], eng=eng,
                    tag="xstage",
                )
                return t

            def w1_piece(kd, p0):
                ln = min(PIECE, H - p0)
                sl = slice(p0, p0 + ln)
                load_cast(
                    w1_tiles[kd][:, sl], w1_d[kd * P : (kd + 1) * P, sl], [P, ln]
                )

            # w2 row-sums, column m = rowsum over free dim of w2 m-tile
            w2sum = persist.tile([P, MH], mmdt)

            def w2_group(ms):
                # stream one m-group's worth of w2 and reduce on DVE
                for m in ms:
                    w2t = w2p.tile([P, n_b2], f32)
                    nc.sync.dma_start(out=w2t[:], in_=w2_d[m * P : (m + 1) * P, :])
                    with nc.allow_low_precision(
                        reason="w2 row-sum feeds reduced-precision matmul anyway"
                    ):
                        nc.vector.reduce_sum(
                            out=w2sum[:, m : m + 1],
                            in_=w2t[:],
                            axis=mybir.AxisListType.X,
                        )

            w1_tiles = [
                persist.tile([P, H], mmdt, tag=f"w1k{kd}", name=f"w1k{kd}")
                for kd in range(KD)
            ]
            xtg_tiles = [None] * KD
            xtg_tiles[0] = xtg_load(0, nc.vector)
            xtg_tiles[1] = xtg_load(1, nc.scalar)
            for kd in range(KD):
                w1_piece(kd, 0)
            for kd in range(2, KD):
                xtg_tiles[kd] = xtg_load(kd, nc.scalar)
            # interleave the rest of w1 with w2 m-group-wise: group g's w2
            # row-sums are ready roughly when group g's mm1 finishes, so the
            # matvec pipelines into the mm1 loop instead of trailing the
            # whole weight stream
            groups = [range(m0, min(m0 + GRP, MH)) for m0 in range(0, MH, GRP)]
            w2_group(groups[0])
            for gi, p0 in enumerate(range(PIECE, H, PIECE)):
                for kd in range(KD):
                    w1_piece(kd, p0)
                if gi + 1 < len(groups):
                    w2_group(groups[gi + 1])

            b2sum = persist.tile([1, 1], f32)
            nc.vector.reduce_sum(out=b2sum[:], in_=b2_sb[:], axis=mybir.AxisListType.X)

            # hT per chunk: [128, MH * chunk_len], slice m holds h^T m-tile
            ht_tiles = [
                persist.tile([P, MH * ln], mmdt, tag=f"ht{ci}", name=f"ht{ci}")
                for ci, (off, ln) in enumerate(chunks)
            ]

            # --- mm1 + relu + pipelined matvec ---
            # m-groups of GRP (aligned to the w1 DMA pieces) with the kd
            # accumulation loop outer-per-group: GRP open PSUM accumulators,
            # so the Tensor engine starts on group 0 right after xtg and the
            # first w1 piece land. The matvec against the w2 row-sums runs
            # one group behind mm1 (so the in-order PE stream never waits on
            # a w2 reduce), accumulating into per-chunk PSUM banks that stay
            # open across the whole loop.
            pzs = [
                psum_z.tile([1, ln], f32, tag="psz", name="psz")
                for ci, (off, ln) in enumerate(chunks)
            ]

            def matvec_group(ms):
                for ci, (off, ln) in enumerate(chunks):
                    for m in ms:
                        nc.tensor.matmul(
                            pzs[ci][:],
                            w2sum[:, m : m + 1],
                            ht_tiles[ci][:, m * ln : (m + 1) * ln],
                            start=(m == 0),
                            stop=(m == MH - 1),
                            skip_group_check=True,
                        )

            for gi, grp in enumerate(groups):
                for ci, (off, ln) in enumerate(chunks):
                    pss = {}
                    for m in grp:
                        pss[m] = psum_h.tile([P, ln], f32, tag="psh", name="psh")
                    for kd in range(KD):
                        for m in grp:
                            nc.tensor.matmul(
                                pss[m][:],
                                w1_tiles[kd][:, m * P : (m + 1) * P],
                                xtg_tiles[kd][:, off : off + ln],
                                start=(kd == 0),
                                stop=(kd == KD - 1),
                                skip_group_check=True,
                            )
                    for m in grp:
                        nc.scalar.activation(
                            ht_tiles[ci][:, m * ln : (m + 1) * ln],
                            pss[m][:],
                            mybir.ActivationFunctionType.Relu,
                            bias=b1t_sb[:, m : m + 1],
                        )
                if gi > 0:
                    matvec_group(groups[gi - 1])
            matvec_group(groups[-1])

            # --- bias + gate scale on the accumulated matvec ---
            z_sb = persist.tile([1, C], f32)
            for ci, (off, ln) in enumerate(chunks):
                nc.scalar.activation(
                    z_sb[:, off : off + ln],
                    pzs[ci][:],
                    mybir.ActivationFunctionType.Identity,
                    bias=b2sum[:],
                )
                nc.vector.tensor_mul(
                    z_sb[:, off : off + ln],
                    z_sb[:, off : off + ln],
                    g_sb[:, off : off + ln],
                )
            nc.sync.dma_start(out=z_d[:], in_=z_sb[:])

    nc.compile()
    _BUILD_CACHE[key] = nc
    return nc


def kernel(x, wg, w1, b1, w2, b2, k):
    from concourse.bass_utils import run_bass_kernel_spmd

    x = np.asarray(x)
    wg = np.asarray(wg)
    w1 = np.asarray(w1)
    b1 = np.asarray(b1)
    w2 = np.asarray(w2)
    b2 = np.asarray(b2)
    k = int(k)

    B, S, D = x.shape
    E = wg.shape[1]
    H = w1.shape[2]
    T = B * S
    assert E == N_CORES, f"expert-parallel layout assumes E == 8, got {E}"
    assert D % P == 0 and H % P == 0, (D, H)

    xf = np.ascontiguousarray(x.reshape(T, D), dtype=np.float32)

    # --- gate + top-k routing (host; needed to build the dispatch shards) ---
    logits = xf @ wg.astype(np.float32)
    logits -= logits.max(axis=1, keepdims=True)
    np.exp(logits, out=logits)
    scores = logits / logits.sum(axis=1, keepdims=True)
    if k >= E:
        topi = np.broadcast_to(np.arange(E, dtype=np.int64), (T, E))
    else:
        topi = np.argpartition(-scores, k, axis=1)[:, :k]
    rows = np.arange(T)[:, None]
    topv = scores[rows, topi]

    # per-expert token lists
    idx_e = []
    val_e = []
    for e in range(E):
        tmask, kpos = np.nonzero(topi == e)
        idx_e.append(tmask)
        val_e.append(topv[tmask, kpos].astype(np.float32))
    max_cnt = max(len(i) for i in idx_e)
    C = max(512, _round_up(max_cnt, 128))

    nc = _build_program(D, H, C, w2.shape[2])

    in_maps = []
    for e in range(E):
        n_e = len(idx_e[e])
        xtg = np.zeros((D, C), dtype=np.float32)
        xtg[:, :n_e] = xf[idx_e[e]].T
        g = np.zeros((1, C), dtype=np.float32)
        g[0, :n_e] = val_e[e]
        b1t = np.ascontiguousarray(
            b1[e].astype(np.float32).reshape(H // P, P).T
        )
        in_maps.append(
            {
                "xtg": xtg,
                "w1": np.ascontiguousarray(w1[e], dtype=np.float32),
                "b1t": b1t,
                "w2": np.ascontiguousarray(w2[e], dtype=np.float32),
                "b2": np.ascontiguousarray(b2[e][None, :], dtype=np.float32),
                "g": g,
            }
        )

    res = run_bass_kernel_spmd(nc, in_maps, core_ids=list(range(N_CORES)))

    # --- combine: scatter-add per-(token, expert) scalars, then log_softmax ---
    s = np.zeros(T, dtype=np.float32)
    for e in range(E):
        n_e = len(idx_e[e])
        if n_e:
            s[idx_e[e]] += res.results[e]["z"][0, :n_e]

    sm = s.reshape(B, S)
    sm = sm - sm.max(axis=1, keepdims=True)
    out = sm - np.log(np.exp(sm).sum(axis=1, keepdims=True))
    return out.astype(np.float32)
